# revision 1
# baseline (speedup 1.0000x reference)
"""Gated dual-score (semantic+geometric/RoPE) causal attention layer on 8 TRN2 cores.

Sharding: data-parallel over batch (2) x tensor-parallel over heads (16 -> 4/core).
Core i: batch b = i // 4, heads hg = i % 4 -> heads [4*hg, 4*hg+4).
Each core computes a partial y (its heads' contribution, its batch); the host
sums the 4 partials per batch (the "all-reduce" of the row-sharded out-proj).

On-device layout trick: all projections consume xT (d on partitions, t free,
pre-transposed on host) and produce qT/kT in (d, t) layout. Scores are computed
transposed, (s on partitions, t free), which makes:
  - sem+geo fusion a single 128-contraction matmul (stacked [sem64|geo64] dims),
  - the gate fold into a pre-scale of qT columns,
  - P@V consume the exp tile directly with V in natural (t, dv) layout,
  - the softmax denominator a ones-vector matmul.
Softmax skips max-subtraction (scores are O(5) by construction; fp32/bf16 safe).
Compute dtype bf16 (fp32 matmul costs 2x cycles on TRN2), fp32 accumulation.
"""

import sys
from contextlib import ExitStack

import numpy as np

sys.path.insert(0, "/opt/trn_rl_repo")

import ml_dtypes  # noqa: E402

import concourse.bass as bass  # noqa: E402
from concourse import bacc  # noqa: E402
import concourse.mybir as mybir  # noqa: E402
import concourse.tile as tile  # noqa: E402
from concourse.bass_utils import run_bass_kernel_spmd  # noqa: E402

B, T, D, H = 2, 2048, 2048, 16
SEM_HD = GEO_HD = 64
V_HD = 128
HL = 4  # heads per core
CL = HL * V_HD  # local v-dim (512)
ROPE_BASE = 10000.0
NEG_INF = -1e9

KT = D // 128  # 16 k-tiles over the contraction dim
TT = T // 128  # 16 token tiles of 128
TC = T // 512  # 4 token chunks of 512
BF = mybir.dt.bfloat16
F32 = mybir.dt.float32
NPBF = ml_dtypes.bfloat16

_CACHED_NC = None


def _build_nc():
    nc = bacc.Bacc()

    xt_d = nc.declare_dram_parameter("xt", [KT, 128, T], BF, isOutput=False)
    wq_d = nc.declare_dram_parameter("wq", [HL, 128, KT, 128], BF, isOutput=False)
    wk_d = nc.declare_dram_parameter("wk", [HL, 128, KT, 128], BF, isOutput=False)
    wv_d = nc.declare_dram_parameter("wv", [KT, 128, CL], BF, isOutput=False)
    wo_d = nc.declare_dram_parameter("wo", [HL, 128, D], BF, isOutput=False)
    wg_d = nc.declare_dram_parameter("wg", [128, KT, 2 * HL], BF, isOutput=False)
    glog_d = nc.declare_dram_parameter("glog", [2 * HL, 1], F32, isOutput=False)
    crep_d = nc.declare_dram_parameter("crep", [128, T], BF, isOutput=False)
    srep_d = nc.declare_dram_parameter("srep", [128, T], BF, isOutput=False)
    maskd_d = nc.declare_dram_parameter("maskd", [128, 128], BF, isOutput=False)
    selc_d = nc.declare_dram_parameter("selc", [2 * HL, HL * 128], BF, isOutput=False)
    gsv_d = nc.declare_dram_parameter("gsv", [2 * HL, 1], F32, isOutput=False)
    gbv_d = nc.declare_dram_parameter("gbv", [2 * HL, 1], F32, isOutput=False)
    y_d = nc.declare_dram_parameter("y", [T, D], BF, isOutput=True)

    with tile.TileContext(nc) as tc, ExitStack() as ctx:
        singles = ctx.enter_context(tc.tile_pool(name="singles", bufs=1))
        xpool = ctx.enter_context(tc.tile_pool(name="xpool", bufs=1))
        vpool = ctx.enter_context(tc.tile_pool(name="vpool", bufs=1))
        wqk_pool = ctx.enter_context(tc.tile_pool(name="wqk", bufs=2))
        qk_pool = ctx.enter_context(tc.tile_pool(name="qk", bufs=2))
        ot_pool = ctx.enter_context(tc.tile_pool(name="ot", bufs=1))
        wo_pool = ctx.enter_context(tc.tile_pool(name="wo", bufs=1))
        p_pool = ctx.enter_context(tc.tile_pool(name="pp", bufs=8))
        sc_pool = ctx.enter_context(tc.tile_pool(name="scratch", bufs=2))
        y_pool = ctx.enter_context(tc.tile_pool(name="ysb", bufs=4))

        ps_a = ctx.enter_context(tc.tile_pool(name="ps_a", bufs=3, space="PSUM"))
        ps_s = ctx.enter_context(tc.tile_pool(name="ps_s", bufs=2, space="PSUM"))
        ps_o = ctx.enter_context(tc.tile_pool(name="ps_o", bufs=2, space="PSUM"))
        ps_d = ctx.enter_context(tc.tile_pool(name="ps_d", bufs=1, space="PSUM"))
        ps_g = ps_s

        # ---- x^T chunk 0 first: the first matmuls need it ----
        xt = xpool.tile([128, KT, T], BF)
        for k in range(KT):
            nc.sync.dma_start(out=xt[:, k, 0:512], in_=xt_d[k, :, 0:512])

        # ---- static tables / constants ----
        crep = singles.tile([128, T], BF)
        srep = singles.tile([128, T], BF)
        maskd = singles.tile([128, 128], BF)
        glog = singles.tile([2 * HL, 1], F32)
        nc.sync.dma_start(out=glog, in_=glog_d[:])

        ones_col = singles.tile([128, 1], BF)  # denom lhsT
        nc.vector.memset(ones_col, 1.0)
        one_row = singles.tile([1, 128], BF)  # denom-bcast lhsT
        nc.vector.memset(one_row, 1.0)
        # per-head gate-broadcast selector: rows 0:4 pick g/8 into psum rows
        # 0:64, rows 4:8 pick (1-g)/8 into psum rows 64:128 (host-built)
        selc = singles.tile([2 * HL, HL * 128], BF)
        nc.sync.dma_start(out=selc, in_=selc_d[:])
        gsv = singles.tile([2 * HL, 1], F32)  # [1/8 x4; -1/8 x4]
        gbv = singles.tile([2 * HL, 1], F32)  # [0 x4; 1/8 x4]
        nc.sync.dma_start(out=gsv, in_=gsv_d[:])
        nc.sync.dma_start(out=gbv, in_=gbv_d[:])

        wg = singles.tile([128, KT, 2 * HL], BF)
        nc.sync.dma_start(out=wg, in_=wg_d[:])

        wv_pool = ctx.enter_context(tc.tile_pool(name="wvpool", bufs=1))
        wv = wv_pool.tile([128, KT, CL], BF)
        for k in range(KT):
            nc.sync.dma_start(out=wv[:, k, :], in_=wv_d[k])

        # head-0 q/k weights prefetch ahead of the bulk of xT
        wqk0 = []
        for nm, dparam in (("wq", wq_d), ("wk", wk_d)):
            wsb = wqk_pool.tile([128, KT, 128], BF, tag=nm, name=f"{nm}0")
            nc.sync.dma_start(out=wsb, in_=dparam[0])
            wqk0.append(wsb)

        for k in range(KT):
            nc.sync.dma_start(out=xt[:, k, 512:1024], in_=xt_d[k, :, 512:1024])

        nc.sync.dma_start(out=crep, in_=crep_d[:])
        nc.sync.dma_start(out=srep, in_=srep_d[:])
        nc.sync.dma_start(out=maskd, in_=maskd_d[:])

        # rest of x^T
        for j in range(2, TC):
            for k in range(KT):
                nc.sync.dma_start(
                    out=xt[:, k, 512 * j : 512 * (j + 1)],
                    in_=xt_d[k, :, 512 * j : 512 * (j + 1)],
                )

        # ---- gate projection: rows 0:4 and 4:8 both sigmoid(glog + x@gate_w);
        # gcomb rows 0:4 = g/8, rows 4:8 = (1-g)/8, fp32 (8, T)
        gcomb = singles.tile([2 * HL, T], BF)
        for j in range(TC):
            tsl = slice(512 * j, 512 * (j + 1))
            pg = ps_g.tile([2 * HL, 512], F32, tag="ps", name="pg")
            for k in range(KT):
                nc.tensor.matmul(
                    pg, wg[:, k, :], xt[:, k, tsl], start=(k == 0), stop=(k == KT - 1)
                )
            gsig = sc_pool.tile([2 * HL, 512], F32, tag="gsig", bufs=1)
            nc.scalar.activation(
                gsig, pg, mybir.ActivationFunctionType.Sigmoid, bias=glog
            )
            nc.scalar.activation(
                gcomb[:, tsl],
                gsig,
                mybir.ActivationFunctionType.Identity,
                scale=gsv,
                bias=gbv,
            )

        # ---- V projection, natural (t, dv) layout: lhsT = xT tile, rhs = w_v ----
        v_sb = vpool.tile([128, TT, CL], BF)
        for i in range(TT):
            pv = ps_a.tile([128, CL], F32, tag="big", name="pv")
            for k in range(KT):
                nc.tensor.matmul(
                    pv,
                    xt[:, k, 128 * i : 128 * (i + 1)],
                    wv[:, k, :],
                    start=(k == 0),
                    stop=(k == KT - 1),
                )
            nc.scalar.copy(v_sb[:, i, :], pv)

        # ---- per-head: QK projection (+gate/RoPE fusion) then attention ----
        outT = [
            ot_pool.tile([128, T], BF, tag=f"ot{h}", name=f"ot{h}") for h in range(HL)
        ]
        for h in range(HL):
            if h == 0:
                wq_sb, wk_sb = wqk0
            else:
                wq_sb = wqk_pool.tile([128, KT, 128], BF, tag="wq")
                wk_sb = wqk_pool.tile([128, KT, 128], BF, tag="wk")
                nc.sync.dma_start(out=wq_sb, in_=wq_d[h])
                nc.sync.dma_start(out=wk_sb, in_=wk_d[h])
            qstk = qk_pool.tile([128, T], BF, tag="qstk")
            kstk = qk_pool.tile([128, T], BF, tag="kstk")

            for j in range(TC):
                tsl = slice(512 * j, 512 * (j + 1))
                pq = ps_a.tile([128, 512], F32, tag="big", name="pq")
                pk = ps_a.tile([128, 512], F32, tag="big", name="pk")
                for k in range(KT):
                    nc.tensor.matmul(
                        pq, wq_sb[:, k, :], xt[:, k, tsl],
                        start=(k == 0), stop=(k == KT - 1),
                    )
                for k in range(KT):
                    nc.tensor.matmul(
                        pk, wk_sb[:, k, :], xt[:, k, tsl],
                        start=(k == 0), stop=(k == KT - 1),
                    )
                # gate broadcast: rows 0:64 <- g/8, rows 64:128 <- (1-g)/8
                gbb = ps_g.tile([128, 512], F32, tag="ps", name="gbb")
                nc.tensor.matmul(
                    gbb,
                    selc[:, 128 * h : 128 * (h + 1)],
                    gcomb[:, tsl],
                    start=True,
                    stop=True,
                )

                gbs = sc_pool.tile([128, 512], F32, tag="gbs", bufs=2)
                nc.scalar.copy(gbs, gbb)
                # q: sem rows scaled by g/8
                nc.vector.tensor_mul(qstk[0:64, tsl], pq[0:64, :], gbs[0:64, :])
                # q geo: rotate by RoPE then scale by (1-g)/8
                m1 = sc_pool.tile([128, 512], BF, tag="m1", bufs=4)
                m2 = sc_pool.tile([128, 512], BF, tag="m2", bufs=4)
                m2b = sc_pool.tile([128, 512], BF, tag="m2", bufs=4)
                nc.vector.tensor_mul(m1[64:128, :], pq[64:128, :], crep[64:128, tsl])
                nc.vector.tensor_mul(m2[64:128, :], pq[64:128, :], srep[64:128, tsl])
                nc.vector.tensor_copy(m2b[64:96, :], m2[96:128, :])
                nc.vector.tensor_copy(m2b[96:128, :], m2[64:96, :])
                nc.vector.tensor_sub(m1[64:96, :], m1[64:96, :], m2b[64:96, :])
                nc.vector.tensor_add(m1[96:128, :], m1[96:128, :], m2b[96:128, :])
                nc.vector.tensor_mul(qstk[64:128, tsl], m1[64:128, :], gbs[64:128, :])
                # k: sem rows copied, geo rows rotated (scale folded into q side)
                nc.scalar.copy(kstk[0:64, tsl], pk[0:64, :])
                km1 = sc_pool.tile([128, 512], BF, tag="m1", bufs=4)
                km2 = sc_pool.tile([128, 512], BF, tag="m2", bufs=4)
                km2b = sc_pool.tile([128, 512], BF, tag="m2", bufs=4)
                nc.vector.tensor_mul(km1[64:128, :], pk[64:128, :], crep[64:128, tsl])
                nc.vector.tensor_mul(km2[64:128, :], pk[64:128, :], srep[64:128, tsl])
                nc.vector.tensor_copy(km2b[64:96, :], km2[96:128, :])
                nc.vector.tensor_copy(km2b[96:128, :], km2[64:96, :])
                nc.vector.tensor_sub(kstk[64:96, tsl], km1[64:96, :], km2b[64:96, :])
                nc.vector.tensor_add(kstk[96:128, tsl], km1[96:128, :], km2b[96:128, :])

            # attention for this head, chunk by chunk
            for j in range(TC):
                tsl = slice(512 * j, 512 * (j + 1))
                po = ps_o.tile([128, 512], F32, tag="po")
                pd = ps_d.tile([1, 512], F32, tag="pd")
                n_s = 4 * (j + 1)
                for s in range(n_s):
                    dj = s - 4 * j  # >=0 on diagonal tiles
                    c0 = 128 * dj if dj >= 0 else 0
                    ssl = slice(128 * s, 128 * (s + 1))
                    ps = ps_s.tile([128, 512], F32, tag="ps", name="ps")
                    nc.tensor.matmul(
                        ps[:, c0:512],
                        kstk[:, ssl],
                        qstk[:, 512 * j + c0 : 512 * (j + 1)],
                        start=True,
                        stop=True,
                    )
                    pt = p_pool.tile([128, 512], BF, tag="pt", name="pt")
                    nc.scalar.activation(
                        pt[:, c0:512], ps[:, c0:512], mybir.ActivationFunctionType.Exp
                    )
                    if dj >= 0:
                        nc.vector.tensor_mul(
                            pt[:, c0 : c0 + 128], pt[:, c0 : c0 + 128], maskd
                        )
                    nc.tensor.matmul(
                        pd[:, c0:512],
                        ones_col,
                        pt[:, c0:512],
                        start=(s == 0),
                        stop=(s == n_s - 1),
                    )
                    nc.tensor.matmul(
                        po[:, c0:512],
                        v_sb[:, s, 128 * h : 128 * (h + 1)],
                        pt[:, c0:512],
                        start=(s == 0),
                        stop=(s == n_s - 1),
                    )
                # normalize: broadcast denom down partitions (bf16 matmul),
                # then approx-reciprocal the full tile on DVE
                pdb = sc_pool.tile([1, 512], BF, tag="rec")
                nc.scalar.copy(pdb, pd)
                rbc = ps_d.tile([128, 512], F32, tag="pd", name="rbc")
                nc.tensor.matmul(rbc, one_row, pdb, start=True, stop=True)
                rbs = sc_pool.tile([128, 512], F32, tag="rbs")
                nc.vector.reciprocal_approx_fast(out=rbs, in_=rbc)
                nc.vector.tensor_mul(outT[h][:, tsl], po, rbs)

        # ---- out-projection: y[t, e] = sum_h outT_h^T @ wo_h ----
        wo_sb = [
            wo_pool.tile([128, D], BF, tag=f"wo{h}", name=f"wo{h}") for h in range(HL)
        ]
        for h in range(HL):
            nc.sync.dma_start(out=wo_sb[h], in_=wo_d[h])
        for i in range(TT):
            for ec in range(D // 512):
                py = ps_a.tile([128, 512], F32, tag="big", name="py")
                for h in range(HL):
                    nc.tensor.matmul(
                        py,
                        outT[h][:, 128 * i : 128 * (i + 1)],
                        wo_sb[h][:, 512 * ec : 512 * (ec + 1)],
                        start=(h == 0),
                        stop=(h == HL - 1),
                    )
                ysb = y_pool.tile([128, 512], BF, tag="ysb")
                nc.scalar.copy(ysb, py)
                nc.sync.dma_start(
                    out=y_d[128 * i : 128 * (i + 1), 512 * ec : 512 * (ec + 1)],
                    in_=ysb,
                )

    nc.finalize()
    return nc


def _host_prep(x, w_q_sem, w_k_sem, w_q_geo, w_k_geo, w_v, w_out, gate_logit, gate_w):
    """Build the 8 per-core input maps (all numpy, bf16 where matmul-bound)."""
    half = GEO_HD // 2  # 32
    inv_freq = 1.0 / (ROPE_BASE ** (np.arange(half, dtype=np.float64) / half))
    pos = np.arange(T, dtype=np.float64)
    ang = pos[None, :] * inv_freq[:, None]  # (32, T)
    crep = np.zeros((128, T), dtype=NPBF)
    srep = np.zeros((128, T), dtype=NPBF)
    crep[64:96] = np.cos(ang)
    crep[96:128] = np.cos(ang)
    srep[64:96] = np.sin(ang)
    srep[96:128] = np.sin(ang)

    p_i = np.arange(128)
    maskd = np.where(p_i[:, None] <= p_i[None, :], 1.0, 0.0).astype(NPBF)

    # per-head stacked [sem64 | geo64] projection weights, (128, KT, 128) layout
    def stack_heads(wsem, wgeo):
        out = []
        for h in range(H):
            blk = np.concatenate(
                [wsem[:, 64 * h : 64 * (h + 1)], wgeo[:, 64 * h : 64 * (h + 1)]],
                axis=1,
            )  # (D, 128)
            out.append(
                np.ascontiguousarray(
                    blk.reshape(KT, 128, 128).transpose(1, 0, 2)
                ).astype(NPBF)
            )
        return out  # H x (128, KT, 128)

    wq_all = stack_heads(w_q_sem, w_q_geo)
    wk_all = stack_heads(w_k_sem, w_k_geo)

    in_maps = []
    for core in range(8):
        b, hg = core // 4, core % 4
        heads = range(4 * hg, 4 * hg + 4)
        xt = np.ascontiguousarray(x[b].T).astype(NPBF).reshape(KT, 128, T)
        wq = np.stack([wq_all[h] for h in heads])
        wk = np.stack([wk_all[h] for h in heads])
        wv = w_v[:, CL * hg : CL * (hg + 1)].reshape(KT, 128, CL).astype(NPBF)
        wo = w_out[CL * hg : CL * (hg + 1), :].reshape(HL, 128, D).astype(NPBF)
        gwl = gate_w[:, 4 * hg : 4 * hg + 4]  # (D, 4)
        gw2 = np.concatenate([gwl, gwl], axis=1)  # (D, 8) duplicated
        wg = np.ascontiguousarray(
            gw2.reshape(KT, 128, 2 * HL).transpose(1, 0, 2)
        ).astype(NPBF)
        selc = np.zeros((2 * HL, HL * 128), dtype=NPBF)
        for h in range(HL):
            selc[h, 128 * h : 128 * h + 64] = 1.0
            selc[HL + h, 128 * h + 64 : 128 * h + 128] = 1.0
        gsv = np.array([0.125] * HL + [-0.125] * HL, dtype=np.float32).reshape(2 * HL, 1)
        gbv = np.array([0.0] * HL + [0.125] * HL, dtype=np.float32).reshape(2 * HL, 1)
        gll = gate_logit[4 * hg : 4 * hg + 4]
        glog = np.ascontiguousarray(
            np.concatenate([gll, gll]).reshape(2 * HL, 1)
        ).astype(np.float32)
        in_maps.append(
            {
                "xt": xt,
                "wq": wq,
                "wk": wk,
                "wv": np.ascontiguousarray(wv),
                "wo": np.ascontiguousarray(wo),
                "wg": wg,
                "glog": glog,
                "crep": crep,
                "srep": srep,
                "maskd": maskd,
                "selc": selc,
                "gsv": gsv,
                "gbv": gbv,
            }
        )
    return in_maps


def _run(inputs, trace=False):
    global _CACHED_NC
    if _CACHED_NC is None:
        _CACHED_NC = _build_nc()
    in_maps = _host_prep(**{k: np.asarray(v) for k, v in inputs.items()})
    res = run_bass_kernel_spmd(
        _CACHED_NC, in_maps, core_ids=list(range(8)), trace=trace
    )
    y = np.zeros((B, T, D), dtype=np.float32)
    for core in range(8):
        y[core // 4] += res.results[core]["y"].astype(np.float32)
    return y, res


def kernel(**inputs) -> np.ndarray:
    y, _ = _run(inputs, trace=False)
    return y



# revision 4
# speedup vs baseline: 1.1499x; 1.1499x over previous
"""Gated dual-score (semantic+geometric/RoPE) causal attention layer on 8 TRN2 cores.

Sharding: data-parallel over batch (2) x tensor-parallel over heads (16 -> 4/core).
Core i: batch b = i // 4, heads hg = i % 4 -> heads [4*hg, 4*hg+4).
Each core computes a partial y (its heads' contribution, its batch); the host
sums the 4 partials per batch (the "all-reduce" of the row-sharded out-proj).

On-device layout: all projections consume xT (d on partitions, t free) and
produce qT/kT in (d, t) layout. Scores are computed transposed (s on
partitions, t free) so P@V consumes the exp tile directly with V in natural
(t, dv) layout. Key structure choices (all aimed at keeping the PE array,
the bottleneck at ~88% busy, free of non-matmul work):
  - Projections are stacked per head as [q_sem|k_sem] and [k_geo|q_geo] so
    RoPE/gating DVE ops run on full 128-partition tiles (q and k together).
  - The causal mask is folded into the score matmul as a second accumulated
    matmul adding a -1e9 upper-triangular constant (no DVE in exp->PV path).
  - The softmax denominator is accumulated on DVE (bf16 adds of exp tiles)
    and turned into a broadcast row-sum by ONE ones(128x128) matmul per
    chunk, replacing per-tile ones-vector matmuls on the PE.
  - x / weights stream via per-(chunk,k) contiguous DMAs ordered by first
    use on the sync ring; wv/wo/y-stores use the scalar (ACT) ring.
Softmax skips max-subtraction (scores are O(5) by construction).
Compute dtype bf16 (fp32 matmul costs 4x cycles on TRN2), fp32 accumulation.
"""

import sys
from contextlib import ExitStack

import numpy as np

sys.path.insert(0, "/opt/trn_rl_repo")

import ml_dtypes  # noqa: E402

import concourse.bass as bass  # noqa: E402
from concourse import bacc  # noqa: E402
import concourse.mybir as mybir  # noqa: E402
import concourse.tile as tile  # noqa: E402
from concourse.bass_utils import run_bass_kernel_spmd  # noqa: E402

B, T, D, H = 2, 2048, 2048, 16
SEM_HD = GEO_HD = 64
V_HD = 128
HL = 4  # heads per core
CL = HL * V_HD  # local v-dim (512)
ROPE_BASE = 10000.0
NEG_INF = -1e9

KT = D // 128  # 16 k-tiles over the contraction dim
TT = T // 128  # 16 token tiles of 128
TC = T // 512  # 4 token chunks of 512
BF = mybir.dt.bfloat16
F32 = mybir.dt.float32
NPBF = ml_dtypes.bfloat16

_CACHED_NC = None


def _build_nc():
    nc = bacc.Bacc()

    xt_d = nc.declare_dram_parameter("xt", [TC, KT, 128, 512], BF, isOutput=False)
    wsem_d = nc.declare_dram_parameter("wsem", [HL, 128, KT, 128], BF, isOutput=False)
    wgeo_d = nc.declare_dram_parameter("wgeo", [HL, 128, KT, 128], BF, isOutput=False)
    wv_d = nc.declare_dram_parameter("wv", [KT, 128, CL], BF, isOutput=False)
    wo_d = nc.declare_dram_parameter("wo", [HL, 128, D], BF, isOutput=False)
    wg_d = nc.declare_dram_parameter("wg", [128, KT, 2 * HL], BF, isOutput=False)
    glog_d = nc.declare_dram_parameter("glog", [2 * HL, 1], F32, isOutput=False)
    crep_d = nc.declare_dram_parameter("crep", [128, T], BF, isOutput=False)
    srep_d = nc.declare_dram_parameter("srep", [128, T], BF, isOutput=False)
    mbias_d = nc.declare_dram_parameter("mbias", [128, 128], BF, isOutput=False)
    ident_d = nc.declare_dram_parameter("ident", [128, 128], BF, isOutput=False)
    selc_d = nc.declare_dram_parameter("selc", [2 * HL, HL * 128], BF, isOutput=False)
    gsv_d = nc.declare_dram_parameter("gsv", [2 * HL, 1], F32, isOutput=False)
    gbv_d = nc.declare_dram_parameter("gbv", [2 * HL, 1], F32, isOutput=False)
    y_d = nc.declare_dram_parameter("y", [T, D], BF, isOutput=True)

    with tile.TileContext(nc) as tc, ExitStack() as ctx:
        singles = ctx.enter_context(tc.tile_pool(name="singles", bufs=1))
        xpool = ctx.enter_context(tc.tile_pool(name="xpool", bufs=1))
        vpool = ctx.enter_context(tc.tile_pool(name="vpool", bufs=1))
        wqk_pool = ctx.enter_context(tc.tile_pool(name="wqk", bufs=2))
        qk_pool = ctx.enter_context(tc.tile_pool(name="qk", bufs=2))
        ot_pool = ctx.enter_context(tc.tile_pool(name="ot", bufs=1))
        wo_pool = ctx.enter_context(tc.tile_pool(name="wo", bufs=1))
        p_pool = ctx.enter_context(tc.tile_pool(name="pp", bufs=8))
        sc_pool = ctx.enter_context(tc.tile_pool(name="scratch", bufs=2))
        y_pool = ctx.enter_context(tc.tile_pool(name="ysb", bufs=4))

        ps_big = ctx.enter_context(tc.tile_pool(name="ps_big", bufs=3, space="PSUM"))
        ps_s = ctx.enter_context(tc.tile_pool(name="ps_s", bufs=2, space="PSUM"))
        ps_o = ctx.enter_context(tc.tile_pool(name="ps_o", bufs=2, space="PSUM"))
        ps_r = ctx.enter_context(tc.tile_pool(name="ps_r", bufs=1, space="PSUM"))

        # ---- small constants first (the first matmul needs wg) ----
        wg = singles.tile([128, KT, 2 * HL], BF)
        nc.sync.dma_start(out=wg, in_=wg_d[:])
        glog = singles.tile([2 * HL, 1], F32)
        nc.sync.dma_start(out=glog, in_=glog_d[:])
        gsv = singles.tile([2 * HL, 1], F32)  # [1/8 x4; -1/8 x4]
        gbv = singles.tile([2 * HL, 1], F32)  # [0 x4; 1/8 x4]
        nc.sync.dma_start(out=gsv, in_=gsv_d[:])
        nc.sync.dma_start(out=gbv, in_=gbv_d[:])
        selc = singles.tile([2 * HL, HL * 128], BF)
        nc.sync.dma_start(out=selc, in_=selc_d[:])
        ident = singles.tile([128, 128], BF)
        nc.sync.dma_start(out=ident, in_=ident_d[:])
        mbias = singles.tile([128, 128], BF)
        nc.sync.dma_start(out=mbias, in_=mbias_d[:])
        ones128 = singles.tile([128, 128], BF)
        nc.vector.memset(ones128, 1.0)

        # ---- x^T chunk 0: per-k contiguous DMAs so matmuls start ASAP ----
        xt = xpool.tile([128, KT, T], BF)
        for k in range(KT):
            nc.sync.dma_start(out=xt[:, k, 0:512], in_=xt_d[0, k])

        # rope tables (needed right after the first projection chain)
        crep = singles.tile([128, T], BF)
        srep = singles.tile([128, T], BF)
        nc.sync.dma_start(out=crep, in_=crep_d[:])
        nc.sync.dma_start(out=srep, in_=srep_d[:])

        # head-0 q/k weights ahead of the rest of xT
        wsem0 = wqk_pool.tile([128, KT, 128], BF, tag="wsem", name="wsem0")
        wgeo0 = wqk_pool.tile([128, KT, 128], BF, tag="wgeo", name="wgeo0")
        nc.sync.dma_start(out=wsem0, in_=wsem_d[0])
        nc.sync.dma_start(out=wgeo0, in_=wgeo_d[0])

        for j in range(1, TC):
            for k in range(KT):
                nc.sync.dma_start(out=xt[:, k, 512 * j : 512 * (j + 1)], in_=xt_d[j, k])

        # big weights on the ACT ring (keeps sync ring for x / per-head w)
        wv_pool = ctx.enter_context(tc.tile_pool(name="wvpool", bufs=1))
        wv = wv_pool.tile([128, KT, CL], BF)
        for k in range(KT):
            nc.scalar.dma_start(out=wv[:, k, :], in_=wv_d[k])
        wo_sb = [
            wo_pool.tile([128, D], BF, tag=f"wo{h}", name=f"wo{h}") for h in range(HL)
        ]
        for h in range(HL):
            nc.scalar.dma_start(out=wo_sb[h], in_=wo_d[h])

        # ---- helpers ----
        gcomb = singles.tile([2 * HL, T], BF)  # rows 0:4 g/8, rows 4:8 (1-g)/8

        def gate_chunk(j):
            tsl = slice(512 * j, 512 * (j + 1))
            pg = ps_s.tile([2 * HL, 512], F32, tag="ps", name="pg")
            for k in range(KT):
                nc.tensor.matmul(
                    pg, wg[:, k, :], xt[:, k, tsl], start=(k == 0), stop=(k == KT - 1)
                )
            gsig = sc_pool.tile([2 * HL, 512], F32, tag="gsig", bufs=1)
            nc.scalar.activation(
                gsig, pg, mybir.ActivationFunctionType.Sigmoid, bias=glog
            )
            nc.scalar.activation(
                gcomb[:, tsl],
                gsig,
                mybir.ActivationFunctionType.Identity,
                scale=gsv,
                bias=gbv,
            )

        def proj_chunk(h, j, wsem_sb, wgeo_sb, qstk, kstk):
            """QK projection for head h, token chunk j.

            p_sem rows: [q_sem(0:64) | k_sem(64:128)]
            p_geo rows: [k_geo(0:64) | q_geo(64:128)]
            gbs  rows: [g/8   (0:64) | (1-g)/8 (64:128)]  (q-side scales)
            """
            tsl = slice(512 * j, 512 * (j + 1))
            p_sem = ps_big.tile([128, 512], F32, tag="big", name="p_sem")
            p_geo = ps_big.tile([128, 512], F32, tag="big", name="p_geo")
            for k in range(KT):
                nc.tensor.matmul(
                    p_sem, wsem_sb[:, k, :], xt[:, k, tsl],
                    start=(k == 0), stop=(k == KT - 1),
                )
            for k in range(KT):
                nc.tensor.matmul(
                    p_geo, wgeo_sb[:, k, :], xt[:, k, tsl],
                    start=(k == 0), stop=(k == KT - 1),
                )
            gbb = ps_big.tile([128, 512], F32, tag="big", name="gbb")
            nc.tensor.matmul(
                gbb, selc[:, 128 * h : 128 * (h + 1)], gcomb[:, tsl],
                start=True, stop=True,
            )
            gbs = sc_pool.tile([128, 512], BF, tag="gbs", bufs=2)
            nc.scalar.copy(gbs, gbb)

            # RoPE on the stacked geo tile (all 128 partitions per op)
            m1 = sc_pool.tile([128, 512], BF, tag="m1", bufs=2)
            m2 = sc_pool.tile([128, 512], BF, tag="m2", bufs=2)
            sw = sc_pool.tile([128, 512], BF, tag="sw", bufs=2)
            nc.vector.tensor_mul(m1, p_geo, crep[:, tsl])
            nc.vector.tensor_mul(m2, p_geo, srep[:, tsl])
            for blk in range(4):  # swap 32-row halves within each 64
                d0 = 64 * (blk // 2) + 32 * (blk % 2)
                s0 = 64 * (blk // 2) + 32 * (1 - blk % 2)
                nc.vector.tensor_copy(sw[d0 : d0 + 32, :], m2[s0 : s0 + 32, :])
            nc.vector.tensor_add(m1, m1, sw)  # m1 = rotated [k_geo | q_geo]

            # q side gets the gate scales folded in; k side is passthrough
            nc.vector.tensor_mul(qstk[0:64, tsl], p_sem[0:64, :], gbs[0:64, :])
            nc.vector.tensor_mul(qstk[64:128, tsl], m1[64:128, :], gbs[64:128, :])
            nc.vector.tensor_copy(kstk[0:64, tsl], p_sem[64:128, :])
            nc.vector.tensor_copy(kstk[64:128, tsl], m1[0:64, :])

        # ---- gate + head-0 projection, chunk by chunk (tracks DMA arrival) ----
        qstk0 = qk_pool.tile([128, T], BF, tag="qstk", name="qstk0")
        kstk0 = qk_pool.tile([128, T], BF, tag="kstk", name="kstk0")
        for j in range(TC):
            gate_chunk(j)
            proj_chunk(0, j, wsem0, wgeo0, qstk0, kstk0)

        # ---- V projection, natural (t, dv) layout ----
        v_sb = vpool.tile([128, TT, CL], BF)
        for i in range(TT):
            pv = ps_big.tile([128, CL], F32, tag="big", name="pv")
            for k in range(KT):
                nc.tensor.matmul(
                    pv,
                    xt[:, k, 128 * i : 128 * (i + 1)],
                    wv[:, k, :],
                    start=(k == 0),
                    stop=(k == KT - 1),
                )
            if i % 2 == 0:
                nc.scalar.copy(v_sb[:, i, :], pv)
            else:
                nc.vector.tensor_copy(v_sb[:, i, :], pv)

        # ---- per-head: attention, then next head's projection ----
        outT = [
            ot_pool.tile([128, T], BF, tag=f"ot{h}", name=f"ot{h}") for h in range(HL)
        ]
        qstk, kstk = qstk0, kstk0
        for h in range(HL):
            for j in range(TC):
                tsl = slice(512 * j, 512 * (j + 1))
                po = ps_o.tile([128, 512], F32, tag="po")
                acc = sc_pool.tile([128, 512], BF, tag="acc", bufs=2)
                n_s = 4 * (j + 1)
                for s in range(n_s):
                    dj = s - 4 * j  # >=0 on diagonal tiles
                    c0 = 128 * dj if dj >= 0 else 0
                    ssl = slice(128 * s, 128 * (s + 1))
                    ps = ps_s.tile([128, 512], F32, tag="ps", name="ps")
                    nc.tensor.matmul(
                        ps[:, c0:512],
                        kstk[:, ssl],
                        qstk[:, 512 * j + c0 : 512 * (j + 1)],
                        start=True,
                        stop=(dj < 0),
                        skip_group_check=(dj >= 0),
                    )
                    if dj >= 0:
                        # causal mask: add -1e9 upper triangle to the diag block
                        nc.tensor.matmul(
                            ps[:, c0 : c0 + 128],
                            ident,
                            mbias,
                            start=False,
                            stop=True,
                            skip_group_check=True,
                        )
                    pt = p_pool.tile([128, 512], BF, tag="pt", name="pt")
                    nc.scalar.activation(
                        pt[:, c0:512], ps[:, c0:512], mybir.ActivationFunctionType.Exp
                    )
                    if s == 0:
                        nc.vector.tensor_copy(acc, pt)
                    else:
                        nc.vector.tensor_add(
                            acc[:, c0:512], acc[:, c0:512], pt[:, c0:512]
                        )
                    nc.tensor.matmul(
                        po[:, c0:512],
                        v_sb[:, s, 128 * h : 128 * (h + 1)],
                        pt[:, c0:512],
                        start=(s == 0),
                        stop=(s == n_s - 1),
                    )
                # denominator: one matmul broadcasts the partition-sum of acc
                rbc = ps_r.tile([128, 512], F32, tag="rbc", name="rbc")
                nc.tensor.matmul(rbc, ones128, acc, start=True, stop=True)
                rbs = sc_pool.tile([128, 512], F32, tag="rbs", bufs=2)
                nc.vector.reciprocal_approx_fast(out=rbs, in_=rbc)
                nc.vector.tensor_mul(outT[h][:, tsl], po, rbs)

            if h + 1 < HL:
                wsem_sb = wqk_pool.tile([128, KT, 128], BF, tag="wsem")
                wgeo_sb = wqk_pool.tile([128, KT, 128], BF, tag="wgeo")
                nc.sync.dma_start(out=wsem_sb, in_=wsem_d[h + 1])
                nc.sync.dma_start(out=wgeo_sb, in_=wgeo_d[h + 1])
                qstk = qk_pool.tile([128, T], BF, tag="qstk")
                kstk = qk_pool.tile([128, T], BF, tag="kstk")
                for j in range(TC):
                    proj_chunk(h + 1, j, wsem_sb, wgeo_sb, qstk, kstk)

        # ---- out-projection: y[t, e] = sum_h outT_h^T @ wo_h ----
        for i in range(TT):
            for ec in range(D // 512):
                py = ps_big.tile([128, 512], F32, tag="big", name="py")
                for h in range(HL):
                    nc.tensor.matmul(
                        py,
                        outT[h][:, 128 * i : 128 * (i + 1)],
                        wo_sb[h][:, 512 * ec : 512 * (ec + 1)],
                        start=(h == 0),
                        stop=(h == HL - 1),
                    )
                ysb = y_pool.tile([128, 512], BF, tag="ysb")
                if (i + ec) % 2 == 0:
                    nc.scalar.copy(ysb, py)
                else:
                    nc.vector.tensor_copy(ysb, py)
                nc.scalar.dma_start(
                    out=y_d[128 * i : 128 * (i + 1), 512 * ec : 512 * (ec + 1)],
                    in_=ysb,
                )

    nc.finalize()
    return nc


def _host_prep(x, w_q_sem, w_k_sem, w_q_geo, w_k_geo, w_v, w_out, gate_logit, gate_w):
    """Build the 8 per-core input maps (all numpy, bf16 where matmul-bound)."""
    half = GEO_HD // 2  # 32
    inv_freq = 1.0 / (ROPE_BASE ** (np.arange(half, dtype=np.float64) / half))
    pos = np.arange(T, dtype=np.float64)
    ang = pos[None, :] * inv_freq[:, None]  # (32, T)
    cos, sin = np.cos(ang), np.sin(ang)
    crep = np.empty((128, T), dtype=NPBF)
    srep = np.empty((128, T), dtype=NPBF)
    for b0 in (0, 64):
        crep[b0 : b0 + 32] = cos
        crep[b0 + 32 : b0 + 64] = cos
        srep[b0 : b0 + 32] = sin  # sw[0:32]=m2[32:64] needs +sin here
        srep[b0 + 32 : b0 + 64] = -sin  # sw[32:64]=m2[0:32] needs -sin here
    # rot[0:32] = p[0:32]*cos - p[32:64]*sin = m1[0:32] + (p[32:64]*srep[32:64])
    # rot[32:64] = p[32:64]*cos + p[0:32]*sin = m1[32:64] + (p[0:32]*srep[0:32])
    # (sw swaps the 32-blocks, so srep rows carry the sign of the *destination*)

    p_i = np.arange(128)
    mbias = np.where(p_i[:, None] <= p_i[None, :], 0.0, NEG_INF).astype(NPBF)
    ident = np.eye(128, dtype=NPBF)

    def stack_heads(wa, wb):
        # per-head (D, 128) = [wa_head | wb_head], as (128, KT, 128) lhsT tiles
        out = []
        for h in range(H):
            blk = np.concatenate(
                [wa[:, 64 * h : 64 * (h + 1)], wb[:, 64 * h : 64 * (h + 1)]], axis=1
            )
            out.append(
                np.ascontiguousarray(
                    blk.reshape(KT, 128, 128).transpose(1, 0, 2)
                ).astype(NPBF)
            )
        return out

    wsem_all = stack_heads(w_q_sem, w_k_sem)  # [q_sem | k_sem]
    wgeo_all = stack_heads(w_k_geo, w_q_geo)  # [k_geo | q_geo]

    in_maps = []
    for core in range(8):
        b, hg = core // 4, core % 4
        heads = range(4 * hg, 4 * hg + 4)
        xt = np.ascontiguousarray(
            x[b].T.reshape(KT, 128, TC, 512).transpose(2, 0, 1, 3)
        ).astype(NPBF)
        wsem = np.stack([wsem_all[h] for h in heads])
        wgeo = np.stack([wgeo_all[h] for h in heads])
        wv = w_v[:, CL * hg : CL * (hg + 1)].reshape(KT, 128, CL).astype(NPBF)
        wo = w_out[CL * hg : CL * (hg + 1), :].reshape(HL, 128, D).astype(NPBF)
        gwl = gate_w[:, 4 * hg : 4 * hg + 4]  # (D, 4)
        gw2 = np.concatenate([gwl, gwl], axis=1)  # (D, 8) duplicated
        wg = np.ascontiguousarray(
            gw2.reshape(KT, 128, 2 * HL).transpose(1, 0, 2)
        ).astype(NPBF)
        selc = np.zeros((2 * HL, HL * 128), dtype=NPBF)
        for h in range(HL):
            selc[h, 128 * h : 128 * h + 64] = 1.0
            selc[HL + h, 128 * h + 64 : 128 * h + 128] = 1.0
        gsv = np.array([0.125] * HL + [-0.125] * HL, dtype=np.float32).reshape(2 * HL, 1)
        gbv = np.array([0.0] * HL + [0.125] * HL, dtype=np.float32).reshape(2 * HL, 1)
        gll = gate_logit[4 * hg : 4 * hg + 4]
        glog = np.ascontiguousarray(
            np.concatenate([gll, gll]).reshape(2 * HL, 1)
        ).astype(np.float32)
        in_maps.append(
            {
                "xt": xt,
                "wsem": wsem,
                "wgeo": wgeo,
                "wv": np.ascontiguousarray(wv),
                "wo": np.ascontiguousarray(wo),
                "wg": wg,
                "glog": glog,
                "crep": crep,
                "srep": srep,
                "mbias": mbias,
                "ident": ident,
                "selc": selc,
                "gsv": gsv,
                "gbv": gbv,
            }
        )
    return in_maps


def _run(inputs, trace=False):
    global _CACHED_NC
    if _CACHED_NC is None:
        _CACHED_NC = _build_nc()
    in_maps = _host_prep(**{k: np.asarray(v) for k, v in inputs.items()})
    res = run_bass_kernel_spmd(
        _CACHED_NC, in_maps, core_ids=list(range(8)), trace=trace
    )
    y = np.zeros((B, T, D), dtype=np.float32)
    for core in range(8):
        y[core // 4] += res.results[core]["y"].astype(np.float32)
    return y, res


def kernel(**inputs) -> np.ndarray:
    y, _ = _run(inputs, trace=False)
    return y


# revision 13
# speedup vs baseline: 1.1936x; 1.0380x over previous
"""Gated dual-score (semantic+geometric/RoPE) causal attention layer on 8 TRN2 cores.

Sharding: data-parallel over batch (2) x tensor-parallel over heads (16 -> 4/core).
Core i: batch b = i // 4, heads hg = i % 4 -> heads [4*hg, 4*hg+4).
Each core computes a partial y (its heads' contribution, its batch); the host
sums the 4 partials per batch (the "all-reduce" of the row-sharded out-proj).

On-device layout: all projections consume xT (d on partitions, t free) and
produce qT/kT in (d, t) layout. Scores are computed transposed (s on
partitions, t free) so P@V consumes the exp tile directly with V in natural
(t, dv) layout. Key structure choices (all aimed at keeping the PE array,
the bottleneck at ~88% busy, free of non-matmul work):
  - Projections are stacked per head as [q_sem|k_sem] and [k_geo|q_geo] so
    RoPE/gating DVE ops run on full 128-partition tiles (q and k together).
  - The causal mask is folded into the score matmul as a second accumulated
    matmul adding a -1e9 upper-triangular constant (no DVE in exp->PV path).
  - The softmax denominator is accumulated on DVE (bf16 adds of exp tiles)
    and turned into a broadcast row-sum by ONE ones(128x128) matmul per
    chunk, replacing per-tile ones-vector matmuls on the PE.
  - x / weights stream via per-(chunk,k) contiguous DMAs ordered by first
    use on the sync ring; wv/wo/y-stores use the scalar (ACT) ring.
Softmax skips max-subtraction (scores are O(5) by construction).
Compute dtype bf16 (fp32 matmul costs 4x cycles on TRN2), fp32 accumulation.
"""

import sys
from contextlib import ExitStack

import numpy as np

sys.path.insert(0, "/opt/trn_rl_repo")

import ml_dtypes  # noqa: E402

import concourse.bass as bass  # noqa: E402
from concourse import bacc  # noqa: E402
import concourse.mybir as mybir  # noqa: E402
import concourse.tile as tile  # noqa: E402
from concourse.bass_utils import run_bass_kernel_spmd  # noqa: E402

B, T, D, H = 2, 2048, 2048, 16
SEM_HD = GEO_HD = 64
V_HD = 128
HL = 4  # heads per core
CL = HL * V_HD  # local v-dim (512)
ROPE_BASE = 10000.0
NEG_INF = -1e9

KT = D // 128  # 16 k-tiles over the contraction dim
TT = T // 128  # 16 token tiles of 128
TC = T // 512  # 4 token chunks of 512
BF = mybir.dt.bfloat16
F32 = mybir.dt.float32
NPBF = ml_dtypes.bfloat16

_CACHED_NC = None


def _build_nc():
    nc = bacc.Bacc()

    # cpack columns: crep | srep | ident | mbias | selc (rows 0:8)
    CP = 2 * T + 128 + 128 + HL * 128
    xt_d = nc.declare_dram_parameter("xt", [128, TC, KT, 512], BF, isOutput=False)
    wsem_d = nc.declare_dram_parameter("wsem", [HL, 128, KT, 128], BF, isOutput=False)
    wgeo_d = nc.declare_dram_parameter("wgeo", [HL, 128, KT, 128], BF, isOutput=False)
    wv_d = nc.declare_dram_parameter("wv", [128, KT, CL], BF, isOutput=False)
    wo_d = nc.declare_dram_parameter("wo", [HL, 128, D], BF, isOutput=False)
    wg_d = nc.declare_dram_parameter("wg", [128, KT, 2 * HL], BF, isOutput=False)
    cpack_d = nc.declare_dram_parameter("cpack", [128, CP], BF, isOutput=False)
    gpack_d = nc.declare_dram_parameter("gpack", [2 * HL, 3], F32, isOutput=False)
    y_d = nc.declare_dram_parameter("y", [T, D], BF, isOutput=True)

    with tile.TileContext(nc) as tc, ExitStack() as ctx:
        singles = ctx.enter_context(tc.tile_pool(name="singles", bufs=1))
        xpool = ctx.enter_context(tc.tile_pool(name="xpool", bufs=1))
        vpool = ctx.enter_context(tc.tile_pool(name="vpool", bufs=1))
        wqk_pool = ctx.enter_context(tc.tile_pool(name="wqk", bufs=2))
        qk_pool = ctx.enter_context(tc.tile_pool(name="qk", bufs=2))
        ot_pool = ctx.enter_context(tc.tile_pool(name="ot", bufs=1))
        wo_pool = ctx.enter_context(tc.tile_pool(name="wo", bufs=1))
        p_pool = ctx.enter_context(tc.tile_pool(name="pp", bufs=8))
        sc_pool = ctx.enter_context(tc.tile_pool(name="scratch", bufs=2))
        y_pool = ctx.enter_context(tc.tile_pool(name="ysb", bufs=2))

        ps_big = ctx.enter_context(tc.tile_pool(name="ps_big", bufs=3, space="PSUM"))
        ps_s = ctx.enter_context(tc.tile_pool(name="ps_s", bufs=2, space="PSUM"))
        ps_o = ctx.enter_context(tc.tile_pool(name="ps_o", bufs=2, space="PSUM"))
        ps_r = ctx.enter_context(tc.tile_pool(name="ps_r", bufs=1, space="PSUM"))

        # ---- sync ring: wg/gpack first (the first matmul needs wg), then xT
        # chunk by chunk; per-head q/k weights follow chunk 0.
        wg = singles.tile([128, KT, 2 * HL], BF)
        nc.sync.dma_start(out=wg, in_=wg_d[:])
        gpack = singles.tile([2 * HL, 3], F32)
        nc.sync.dma_start(out=gpack, in_=gpack_d[:])
        glog = gpack[:, 0:1]
        gsv = gpack[:, 1:2]  # [1/8 x4; -1/8 x4]
        gbv = gpack[:, 2:3]  # [0 x4; 1/8 x4]
        ones128 = singles.tile([128, 128], BF)
        nc.vector.memset(ones128, 1.0)

        xt = xpool.tile([128, TC, KT, 512], BF)
        nc.sync.dma_start(out=xt[:, 0, 0:8], in_=xt_d[:, 0, 0:8])
        nc.sync.dma_start(out=xt[:, 0, 8:KT], in_=xt_d[:, 0, 8:KT])

        wsem0 = wqk_pool.tile([128, KT, 128], BF, tag="wsem", name="wsem0")
        wgeo0 = wqk_pool.tile([128, KT, 128], BF, tag="wgeo", name="wgeo0")
        nc.sync.dma_start(out=wsem0, in_=wsem_d[0])
        nc.sync.dma_start(out=wgeo0, in_=wgeo_d[0])

        for j in range(1, TC):
            nc.sync.dma_start(out=xt[:, j], in_=xt_d[:, j])

        # ACT ring: packed constants (rope tables et al), wv, wo
        cpack = singles.tile([128, CP], BF)
        nc.scalar.dma_start(out=cpack, in_=cpack_d[:])
        crep = cpack[:, 0:T]
        srep = cpack[:, T : 2 * T]
        ident = cpack[:, 2 * T : 2 * T + 128]
        mbias = cpack[:, 2 * T + 128 : 2 * T + 256]
        selc = cpack[0 : 2 * HL, 2 * T + 256 : 2 * T + 256 + HL * 128]

        wv_pool = ctx.enter_context(tc.tile_pool(name="wvpool", bufs=1))
        wv = wv_pool.tile([128, KT, CL], BF)
        nc.scalar.dma_start(out=wv, in_=wv_d[:])
        wo_sb = [
            wo_pool.tile([128, D], BF, tag=f"wo{h}", name=f"wo{h}") for h in range(HL)
        ]
        for h in range(HL):
            nc.scalar.dma_start(out=wo_sb[h], in_=wo_d[h])

        # ---- helpers ----
        gcomb = singles.tile([2 * HL, T], BF)  # rows 0:4 g/8, rows 4:8 (1-g)/8

        def gate_chunk(j):
            tsl = slice(512 * j, 512 * (j + 1))
            pg = ps_s.tile([2 * HL, 512], F32, tag="ps", name="pg")
            for k in range(KT):
                nc.tensor.matmul(
                    pg, wg[:, k, :], xt[:, j, k, :], start=(k == 0), stop=(k == KT - 1)
                )
            gsig = sc_pool.tile([2 * HL, 512], F32, tag="gsig", bufs=1)
            nc.scalar.activation(
                gsig, pg, mybir.ActivationFunctionType.Sigmoid, bias=glog
            )
            nc.scalar.activation(
                gcomb[:, tsl],
                gsig,
                mybir.ActivationFunctionType.Identity,
                scale=gsv,
                bias=gbv,
            )

        def proj_chunk(h, j, wsem_sb, wgeo_sb, qstk, kstk):
            """QK projection for head h, token chunk j.

            p_sem rows: [q_sem(0:64) | k_sem(64:128)]
            p_geo rows: [k_geo(0:64) | q_geo(64:128)]
            gbs  rows: [g/8   (0:64) | (1-g)/8 (64:128)]  (q-side scales)
            """
            tsl = slice(512 * j, 512 * (j + 1))
            p_sem = ps_big.tile([128, 512], F32, tag="big", name="p_sem")
            p_geo = ps_big.tile([128, 512], F32, tag="big", name="p_geo")
            for k in range(KT):
                nc.tensor.matmul(
                    p_sem, wsem_sb[:, k, :], xt[:, j, k, :],
                    start=(k == 0), stop=(k == KT - 1),
                )
            for k in range(KT):
                nc.tensor.matmul(
                    p_geo, wgeo_sb[:, k, :], xt[:, j, k, :],
                    start=(k == 0), stop=(k == KT - 1),
                )
            gbb = ps_big.tile([128, 512], F32, tag="big", name="gbb")
            nc.tensor.matmul(
                gbb, selc[:, 128 * h : 128 * (h + 1)], gcomb[:, tsl],
                start=True, stop=True,
            )
            gbs = sc_pool.tile([128, 512], BF, tag="gbs", bufs=2)
            nc.scalar.copy(gbs, gbb)

            # RoPE on the stacked geo tile (all 128 partitions per op)
            m1 = sc_pool.tile([128, 512], BF, tag="m1", bufs=2)
            m2 = sc_pool.tile([128, 512], BF, tag="m2", bufs=2)
            sw = sc_pool.tile([128, 512], BF, tag="sw", bufs=2)
            nc.vector.tensor_mul(m1, p_geo, crep[:, tsl])
            nc.vector.tensor_mul(m2, p_geo, srep[:, tsl])
            for blk in range(4):  # swap 32-row halves within each 64
                d0 = 64 * (blk // 2) + 32 * (blk % 2)
                s0 = 64 * (blk // 2) + 32 * (1 - blk % 2)
                nc.vector.tensor_copy(sw[d0 : d0 + 32, :], m2[s0 : s0 + 32, :])
            nc.vector.tensor_add(m1, m1, sw)  # m1 = rotated [k_geo | q_geo]

            # q side gets the gate scales folded in; k side is passthrough
            nc.vector.tensor_mul(qstk[0:64, tsl], p_sem[0:64, :], gbs[0:64, :])
            nc.vector.tensor_mul(qstk[64:128, tsl], m1[64:128, :], gbs[64:128, :])
            nc.vector.tensor_copy(kstk[0:64, tsl], p_sem[64:128, :])
            nc.vector.tensor_copy(kstk[64:128, tsl], m1[0:64, :])

        # ---- gate + head-0 projection, chunk by chunk (tracks DMA arrival) ----
        qstk0 = qk_pool.tile([128, T], BF, tag="qstk", name="qstk0")
        kstk0 = qk_pool.tile([128, T], BF, tag="kstk", name="kstk0")
        for j in range(TC):
            gate_chunk(j)
            proj_chunk(0, j, wsem0, wgeo0, qstk0, kstk0)

        # ---- V projection, natural (t, dv) layout ----
        v_sb = vpool.tile([128, TT, CL], BF)
        for i in range(TT):
            pv = ps_big.tile([128, CL], F32, tag="big", name="pv")
            for k in range(KT):
                nc.tensor.matmul(
                    pv,
                    xt[:, i // 4, k, 128 * (i % 4) : 128 * (i % 4 + 1)],
                    wv[:, k, :],
                    start=(k == 0),
                    stop=(k == KT - 1),
                )
            if i % 2 == 0:
                nc.scalar.copy(v_sb[:, i, :], pv)
            else:
                nc.vector.tensor_copy(v_sb[:, i, :], pv)

        # ---- per-head: attention, then next head's projection ----
        outT = [
            ot_pool.tile([128, T], BF, tag=f"ot{h}", name=f"ot{h}") for h in range(HL)
        ]
        qstk, kstk = qstk0, kstk0
        for h in range(HL):
            for j in range(TC):
                tsl = slice(512 * j, 512 * (j + 1))
                po = ps_o.tile([128, 512], F32, tag="po")
                acc = sc_pool.tile([128, 512], BF, tag="acc", bufs=2)
                n_s = 4 * (j + 1)
                for s in range(n_s):
                    dj = s - 4 * j  # >=0 on diagonal tiles
                    c0 = 128 * dj if dj >= 0 else 0
                    ssl = slice(128 * s, 128 * (s + 1))
                    ps = ps_s.tile([128, 512], F32, tag="ps", name="ps")
                    nc.tensor.matmul(
                        ps[:, c0:512],
                        kstk[:, ssl],
                        qstk[:, 512 * j + c0 : 512 * (j + 1)],
                        start=True,
                        stop=(dj < 0),
                        skip_group_check=(dj >= 0),
                    )
                    if dj >= 0:
                        # causal mask: add -1e9 upper triangle to the diag block
                        nc.tensor.matmul(
                            ps[:, c0 : c0 + 128],
                            ident,
                            mbias,
                            start=False,
                            stop=True,
                            skip_group_check=True,
                        )
                    pt = p_pool.tile([128, 512], BF, tag="pt", name="pt")
                    nc.scalar.activation(
                        pt[:, c0:512], ps[:, c0:512], mybir.ActivationFunctionType.Exp
                    )
                    if s == 0:
                        nc.vector.tensor_copy(acc, pt)
                    else:
                        nc.vector.tensor_add(
                            acc[:, c0:512], acc[:, c0:512], pt[:, c0:512]
                        )
                    nc.tensor.matmul(
                        po[:, c0:512],
                        v_sb[:, s, 128 * h : 128 * (h + 1)],
                        pt[:, c0:512],
                        start=(s == 0),
                        stop=(s == n_s - 1),
                    )
                # denominator: one matmul broadcasts the partition-sum of acc
                rbc = ps_r.tile([128, 512], F32, tag="rbc", name="rbc")
                nc.tensor.matmul(rbc, ones128, acc, start=True, stop=True)
                rbs = sc_pool.tile([128, 512], F32, tag="rbs", bufs=2)
                nc.vector.reciprocal_approx_fast(out=rbs, in_=rbc)
                nc.vector.tensor_mul(outT[h][:, tsl], po, rbs)

            if h + 1 < HL:
                wsem_sb = wqk_pool.tile([128, KT, 128], BF, tag="wsem")
                wgeo_sb = wqk_pool.tile([128, KT, 128], BF, tag="wgeo")
                nc.sync.dma_start(out=wsem_sb, in_=wsem_d[h + 1])
                nc.sync.dma_start(out=wgeo_sb, in_=wgeo_d[h + 1])
                qstk = qk_pool.tile([128, T], BF, tag="qstk")
                kstk = qk_pool.tile([128, T], BF, tag="kstk")
                for j in range(TC):
                    proj_chunk(h + 1, j, wsem_sb, wgeo_sb, qstk, kstk)

        # ---- out-projection: y[t, e] = sum_h outT_h^T @ wo_h ----
        for i in range(TT):
            ysb = y_pool.tile([128, D], BF, tag="ysb")
            for ec in range(D // 512):
                py = ps_big.tile([128, 512], F32, tag="big", name="py")
                for h in range(HL):
                    nc.tensor.matmul(
                        py,
                        outT[h][:, 128 * i : 128 * (i + 1)],
                        wo_sb[h][:, 512 * ec : 512 * (ec + 1)],
                        start=(h == 0),
                        stop=(h == HL - 1),
                    )
                if ec % 2 == 0:
                    nc.scalar.copy(ysb[:, 512 * ec : 512 * (ec + 1)], py)
                else:
                    nc.vector.tensor_copy(ysb[:, 512 * ec : 512 * (ec + 1)], py)
            nc.scalar.dma_start(out=y_d[128 * i : 128 * (i + 1), :], in_=ysb)

    nc.finalize()
    return nc


def _host_prep(x, w_q_sem, w_k_sem, w_q_geo, w_k_geo, w_v, w_out, gate_logit, gate_w):
    """Build the 8 per-core input maps (all numpy, bf16 where matmul-bound)."""
    half = GEO_HD // 2  # 32
    inv_freq = 1.0 / (ROPE_BASE ** (np.arange(half, dtype=np.float64) / half))
    pos = np.arange(T, dtype=np.float64)
    ang = pos[None, :] * inv_freq[:, None]  # (32, T)
    cos, sin = np.cos(ang), np.sin(ang)
    crep = np.empty((128, T), dtype=NPBF)
    srep = np.empty((128, T), dtype=NPBF)
    for b0 in (0, 64):
        crep[b0 : b0 + 32] = cos
        crep[b0 + 32 : b0 + 64] = cos
        srep[b0 : b0 + 32] = sin  # sw[0:32]=m2[32:64] needs +sin here
        srep[b0 + 32 : b0 + 64] = -sin  # sw[32:64]=m2[0:32] needs -sin here
    # rot[0:32] = p[0:32]*cos - p[32:64]*sin = m1[0:32] + (p[32:64]*srep[32:64])
    # rot[32:64] = p[32:64]*cos + p[0:32]*sin = m1[32:64] + (p[0:32]*srep[0:32])
    # (sw swaps the 32-blocks, so srep rows carry the sign of the *destination*)

    p_i = np.arange(128)
    mbias = np.where(p_i[:, None] <= p_i[None, :], 0.0, NEG_INF).astype(NPBF)
    ident = np.eye(128, dtype=NPBF)
    selc = np.zeros((128, HL * 128), dtype=NPBF)
    for h in range(HL):
        selc[h, 128 * h : 128 * h + 64] = 1.0
        selc[HL + h, 128 * h + 64 : 128 * h + 128] = 1.0
    cpack = np.concatenate(
        [crep, srep, ident, mbias, selc], axis=1
    )  # (128, 2T+256+512)

    def stack_heads(wa, wb):
        # per-head (D, 128) = [wa_head | wb_head], as (128, KT, 128) lhsT tiles
        out = []
        for h in range(H):
            blk = np.concatenate(
                [wa[:, 64 * h : 64 * (h + 1)], wb[:, 64 * h : 64 * (h + 1)]], axis=1
            )
            out.append(
                np.ascontiguousarray(
                    blk.reshape(KT, 128, 128).transpose(1, 0, 2)
                ).astype(NPBF)
            )
        return out

    wsem_all = stack_heads(w_q_sem, w_k_sem)  # [q_sem | k_sem]
    wgeo_all = stack_heads(w_k_geo, w_q_geo)  # [k_geo | q_geo]

    xt_by_b = [
        np.ascontiguousarray(
            x[b].T.reshape(KT, 128, TC, 512).transpose(1, 2, 0, 3)
        ).astype(NPBF)
        for b in range(B)
    ]  # (128, TC, KT, 512): [p, j, k, c] = xT[128k+p, 512j+c]

    in_maps = []
    for core in range(8):
        b, hg = core // 4, core % 4
        heads = range(4 * hg, 4 * hg + 4)
        wsem = np.stack([wsem_all[h] for h in heads])
        wgeo = np.stack([wgeo_all[h] for h in heads])
        wv = np.ascontiguousarray(
            w_v[:, CL * hg : CL * (hg + 1)].reshape(KT, 128, CL).transpose(1, 0, 2)
        ).astype(NPBF)
        wo = w_out[CL * hg : CL * (hg + 1), :].reshape(HL, 128, D).astype(NPBF)
        gwl = gate_w[:, 4 * hg : 4 * hg + 4]  # (D, 4)
        gw2 = np.concatenate([gwl, gwl], axis=1)  # (D, 8) duplicated
        wg = np.ascontiguousarray(
            gw2.reshape(KT, 128, 2 * HL).transpose(1, 0, 2)
        ).astype(NPBF)
        gll = gate_logit[4 * hg : 4 * hg + 4]
        glog = np.concatenate([gll, gll]).astype(np.float32)
        gsv = np.array([0.125] * HL + [-0.125] * HL, dtype=np.float32)
        gbv = np.array([0.0] * HL + [0.125] * HL, dtype=np.float32)
        gpack = np.ascontiguousarray(np.stack([glog, gsv, gbv], axis=1))  # (8, 3)
        in_maps.append(
            {
                "xt": xt_by_b[b],
                "wsem": wsem,
                "wgeo": wgeo,
                "wv": wv,
                "wo": np.ascontiguousarray(wo),
                "wg": wg,
                "cpack": cpack,
                "gpack": gpack,
            }
        )
    return in_maps


def _run(inputs, trace=False):
    global _CACHED_NC
    if _CACHED_NC is None:
        _CACHED_NC = _build_nc()
    in_maps = _host_prep(**{k: np.asarray(v) for k, v in inputs.items()})
    res = run_bass_kernel_spmd(
        _CACHED_NC, in_maps, core_ids=list(range(8)), trace=trace
    )
    y = np.zeros((B, T, D), dtype=np.float32)
    for core in range(8):
        y[core // 4] += res.results[core]["y"].astype(np.float32)
    return y, res


def kernel(**inputs) -> np.ndarray:
    y, _ = _run(inputs, trace=False)
    return y


# revision 17
# speedup vs baseline: 1.1987x; 1.0042x over previous
"""Gated dual-score (semantic+geometric/RoPE) causal attention layer on 8 TRN2 cores.

Sharding: data-parallel over batch (2) x tensor-parallel over heads (16 -> 4/core).
Core i: batch b = i // 4, heads hg = i % 4 -> heads [4*hg, 4*hg+4).
Each core computes a partial y (its heads' contribution, its batch); the host
sums the 4 partials per batch (the "all-reduce" of the row-sharded out-proj).

On-device layout: all projections consume xT (d on partitions, t free) and
produce qT/kT in (d, t) layout. Scores are computed transposed (s on
partitions, t free) so P@V consumes the exp tile directly with V in natural
(t, dv) layout. Key structure choices (all aimed at keeping the PE array,
the bottleneck at ~88% busy, free of non-matmul work):
  - Projections are stacked per head as [q_sem|k_sem] and [k_geo|q_geo] so
    RoPE/gating DVE ops run on full 128-partition tiles (q and k together).
  - The causal mask is folded into the score matmul as a second accumulated
    matmul adding a -1e9 upper-triangular constant (no DVE in exp->PV path).
  - The softmax denominator is accumulated on DVE (bf16 adds of exp tiles)
    and turned into a broadcast row-sum by ONE ones(128x128) matmul per
    chunk, replacing per-tile ones-vector matmuls on the PE.
  - x / weights stream via per-(chunk,k) contiguous DMAs ordered by first
    use on the sync ring; wv/wo/y-stores use the scalar (ACT) ring.
Softmax skips max-subtraction (scores are O(5) by construction).
Compute dtype bf16 (fp32 matmul costs 4x cycles on TRN2), fp32 accumulation.
"""

import sys
from contextlib import ExitStack

import numpy as np

sys.path.insert(0, "/opt/trn_rl_repo")

import ml_dtypes  # noqa: E402

import concourse.bass as bass  # noqa: E402
from concourse import bacc  # noqa: E402
import concourse.mybir as mybir  # noqa: E402
import concourse.tile as tile  # noqa: E402
from concourse.bass_utils import run_bass_kernel_spmd  # noqa: E402

B, T, D, H = 2, 2048, 2048, 16
SEM_HD = GEO_HD = 64
V_HD = 128
HL = 4  # heads per core
CL = HL * V_HD  # local v-dim (512)
ROPE_BASE = 10000.0
NEG_INF = -1e9

KT = D // 128  # 16 k-tiles over the contraction dim
TT = T // 128  # 16 token tiles of 128
TC = T // 512  # 4 token chunks of 512
BF = mybir.dt.bfloat16
F32 = mybir.dt.float32
NPBF = ml_dtypes.bfloat16

_CACHED_NC = None


def _build_nc():
    nc = bacc.Bacc()

    # cpack columns: crep | srep | ident | mbias | selc (rows 0:8)
    CP = 2 * T + 128 + 128 + HL * 128
    xt_d = nc.declare_dram_parameter("xt", [128, TC, KT, 512], BF, isOutput=False)
    wsem_d = nc.declare_dram_parameter("wsem", [HL, 128, KT, 128], BF, isOutput=False)
    wgeo_d = nc.declare_dram_parameter("wgeo", [HL, 128, KT, 128], BF, isOutput=False)
    wv_d = nc.declare_dram_parameter("wv", [128, KT, CL], BF, isOutput=False)
    wo_d = nc.declare_dram_parameter("wo", [HL, 128, D], BF, isOutput=False)
    wg_d = nc.declare_dram_parameter("wg", [128, KT, 2 * HL], BF, isOutput=False)
    cpack_d = nc.declare_dram_parameter("cpack", [128, CP], BF, isOutput=False)
    gpack_d = nc.declare_dram_parameter("gpack", [2 * HL, 3], F32, isOutput=False)
    y_d = nc.declare_dram_parameter("y", [T, D], BF, isOutput=True)

    with tile.TileContext(nc) as tc, ExitStack() as ctx:
        singles = ctx.enter_context(tc.tile_pool(name="singles", bufs=1))
        xpool = ctx.enter_context(tc.tile_pool(name="xpool", bufs=1))
        vpool = ctx.enter_context(tc.tile_pool(name="vpool", bufs=1))
        wqk_pool = ctx.enter_context(tc.tile_pool(name="wqk", bufs=2))
        qk_pool = ctx.enter_context(tc.tile_pool(name="qk", bufs=2))
        ot_pool = ctx.enter_context(tc.tile_pool(name="ot", bufs=1))
        wo_pool = ctx.enter_context(tc.tile_pool(name="wo", bufs=1))
        p_pool = ctx.enter_context(tc.tile_pool(name="pp", bufs=8))
        sc_pool = ctx.enter_context(tc.tile_pool(name="scratch", bufs=2))
        y_pool = ctx.enter_context(tc.tile_pool(name="ysb", bufs=2))

        ps_big = ctx.enter_context(tc.tile_pool(name="ps_big", bufs=3, space="PSUM"))
        ps_s = ctx.enter_context(tc.tile_pool(name="ps_s", bufs=2, space="PSUM"))
        ps_o = ctx.enter_context(tc.tile_pool(name="ps_o", bufs=2, space="PSUM"))
        ps_r = ctx.enter_context(tc.tile_pool(name="ps_r", bufs=1, space="PSUM"))

        # ---- sync ring: wg/gpack first (the first matmul needs wg), then xT
        # chunk by chunk; per-head q/k weights follow chunk 0.
        wg = singles.tile([128, KT, 2 * HL], BF)
        nc.sync.dma_start(out=wg, in_=wg_d[:])
        gpack = singles.tile([2 * HL, 3], F32)
        nc.sync.dma_start(out=gpack, in_=gpack_d[:])
        glog = gpack[:, 0:1]
        gsv = gpack[:, 1:2]  # [1/8 x4; -1/8 x4]
        gbv = gpack[:, 2:3]  # [0 x4; 1/8 x4]
        ones128 = singles.tile([128, 128], BF)
        nc.vector.memset(ones128, 1.0)

        xt = xpool.tile([128, TC, KT, 512], BF)
        for q in range(4):
            nc.sync.dma_start(out=xt[:, 0, 4 * q : 4 * q + 4], in_=xt_d[:, 0, 4 * q : 4 * q + 4])

        wsem0 = wqk_pool.tile([128, KT, 128], BF, tag="wsem", name="wsem0")
        wgeo0 = wqk_pool.tile([128, KT, 128], BF, tag="wgeo", name="wgeo0")
        nc.sync.dma_start(out=wsem0, in_=wsem_d[0])
        nc.sync.dma_start(out=wgeo0, in_=wgeo_d[0])

        # packed constants (rope tables et al) after head-0 weights: needed
        # only once head-0's first projection chain completes
        cpack = singles.tile([128, CP], BF)
        nc.sync.dma_start(out=cpack, in_=cpack_d[:])
        crep = cpack[:, 0:T]
        srep = cpack[:, T : 2 * T]
        ident = cpack[:, 2 * T : 2 * T + 128]
        mbias = cpack[:, 2 * T + 128 : 2 * T + 256]
        selc = cpack[0 : 2 * HL, 2 * T + 256 : 2 * T + 256 + HL * 128]

        for j in range(1, 3):
            nc.sync.dma_start(out=xt[:, j], in_=xt_d[:, j])

        # late-needed bulk on the ACT ring (its early slot is otherwise idle)
        nc.scalar.dma_start(out=xt[:, 3], in_=xt_d[:, 3])
        wv_pool = ctx.enter_context(tc.tile_pool(name="wvpool", bufs=1))
        wv = wv_pool.tile([128, KT, CL], BF)
        nc.scalar.dma_start(out=wv, in_=wv_d[:])
        wo_sb = [
            wo_pool.tile([128, D], BF, tag=f"wo{h}", name=f"wo{h}") for h in range(HL)
        ]

        # ---- helpers ----
        gcomb = singles.tile([2 * HL, T], BF)  # rows 0:4 g/8, rows 4:8 (1-g)/8

        def gate_chunk(j):
            tsl = slice(512 * j, 512 * (j + 1))
            pg = ps_s.tile([2 * HL, 512], F32, tag="ps", name="pg")
            for k in range(KT):
                nc.tensor.matmul(
                    pg, wg[:, k, :], xt[:, j, k, :], start=(k == 0), stop=(k == KT - 1)
                )
            gsig = sc_pool.tile([2 * HL, 512], F32, tag="gsig", bufs=1)
            nc.scalar.activation(
                gsig, pg, mybir.ActivationFunctionType.Sigmoid, bias=glog
            )
            nc.scalar.activation(
                gcomb[:, tsl],
                gsig,
                mybir.ActivationFunctionType.Identity,
                scale=gsv,
                bias=gbv,
            )

        def proj_chunk(h, j, wsem_sb, wgeo_sb, qstk, kstk):
            """QK projection for head h, token chunk j.

            p_sem rows: [q_sem(0:64) | k_sem(64:128)]
            p_geo rows: [k_geo(0:64) | q_geo(64:128)]
            gbs  rows: [g/8   (0:64) | (1-g)/8 (64:128)]  (q-side scales)
            """
            tsl = slice(512 * j, 512 * (j + 1))
            p_sem = ps_big.tile([128, 512], F32, tag="big", name="p_sem")
            p_geo = ps_big.tile([128, 512], F32, tag="big", name="p_geo")
            for k in range(KT):
                nc.tensor.matmul(
                    p_sem, wsem_sb[:, k, :], xt[:, j, k, :],
                    start=(k == 0), stop=(k == KT - 1),
                )
            for k in range(KT):
                nc.tensor.matmul(
                    p_geo, wgeo_sb[:, k, :], xt[:, j, k, :],
                    start=(k == 0), stop=(k == KT - 1),
                )
            gbb = ps_big.tile([128, 512], F32, tag="big", name="gbb")
            nc.tensor.matmul(
                gbb, selc[:, 128 * h : 128 * (h + 1)], gcomb[:, tsl],
                start=True, stop=True,
            )
            gbs = sc_pool.tile([128, 512], BF, tag="gbs", bufs=2)
            nc.scalar.copy(gbs, gbb)

            # RoPE on the stacked geo tile (all 128 partitions per op)
            m1 = sc_pool.tile([128, 512], BF, tag="m1", bufs=2)
            m2 = sc_pool.tile([128, 512], BF, tag="m2", bufs=2)
            sw = sc_pool.tile([128, 512], BF, tag="sw", bufs=2)
            nc.vector.tensor_mul(m1, p_geo, crep[:, tsl])
            nc.vector.tensor_mul(m2, p_geo, srep[:, tsl])
            for blk in range(4):  # swap 32-row halves within each 64
                d0 = 64 * (blk // 2) + 32 * (blk % 2)
                s0 = 64 * (blk // 2) + 32 * (1 - blk % 2)
                nc.vector.tensor_copy(sw[d0 : d0 + 32, :], m2[s0 : s0 + 32, :])
            nc.vector.tensor_add(m1, m1, sw)  # m1 = rotated [k_geo | q_geo]

            # q side gets the gate scales folded in; k side is passthrough
            nc.vector.tensor_mul(qstk[0:64, tsl], p_sem[0:64, :], gbs[0:64, :])
            nc.vector.tensor_mul(qstk[64:128, tsl], m1[64:128, :], gbs[64:128, :])
            nc.vector.tensor_copy(kstk[0:64, tsl], p_sem[64:128, :])
            nc.vector.tensor_copy(kstk[64:128, tsl], m1[0:64, :])

        # ---- gate + head-0 projection, chunk by chunk (tracks DMA arrival) ----
        qstk0 = qk_pool.tile([128, T], BF, tag="qstk", name="qstk0")
        kstk0 = qk_pool.tile([128, T], BF, tag="kstk", name="kstk0")
        for j in range(TC):
            gate_chunk(j)
            proj_chunk(0, j, wsem0, wgeo0, qstk0, kstk0)

        # ---- V projection, natural (t, dv) layout ----
        v_sb = vpool.tile([128, TT, CL], BF)
        for i in range(TT):
            pv = ps_big.tile([128, CL], F32, tag="big", name="pv")
            for k in range(KT):
                nc.tensor.matmul(
                    pv,
                    xt[:, i // 4, k, 128 * (i % 4) : 128 * (i % 4 + 1)],
                    wv[:, k, :],
                    start=(k == 0),
                    stop=(k == KT - 1),
                )
            if i % 2 == 0:
                nc.scalar.copy(v_sb[:, i, :], pv)
            else:
                nc.vector.tensor_copy(v_sb[:, i, :], pv)

        # ---- per-head: attention, then next head's projection ----
        outT = [
            ot_pool.tile([128, T], BF, tag=f"ot{h}", name=f"ot{h}") for h in range(HL)
        ]
        qstk, kstk = qstk0, kstk0
        for h in range(HL):
            for j in range(TC):
                tsl = slice(512 * j, 512 * (j + 1))
                po = ps_o.tile([128, 512], F32, tag="po")
                acc = sc_pool.tile([128, 512], BF, tag="acc", bufs=2)
                n_s = 4 * (j + 1)
                for s in range(n_s):
                    dj = s - 4 * j  # >=0 on diagonal tiles
                    c0 = 128 * dj if dj >= 0 else 0
                    ssl = slice(128 * s, 128 * (s + 1))
                    ps = ps_s.tile([128, 512], F32, tag="ps", name="ps")
                    nc.tensor.matmul(
                        ps[:, c0:512],
                        kstk[:, ssl],
                        qstk[:, 512 * j + c0 : 512 * (j + 1)],
                        start=True,
                        stop=(dj < 0),
                        skip_group_check=(dj >= 0),
                    )
                    if dj >= 0:
                        # causal mask: add -1e9 upper triangle to the diag block
                        nc.tensor.matmul(
                            ps[:, c0 : c0 + 128],
                            ident,
                            mbias,
                            start=False,
                            stop=True,
                            skip_group_check=True,
                        )
                    pt = p_pool.tile([128, 512], BF, tag="pt", name="pt")
                    nc.scalar.activation(
                        pt[:, c0:512], ps[:, c0:512], mybir.ActivationFunctionType.Exp
                    )
                    if s == 0:
                        nc.vector.tensor_copy(acc, pt)
                    elif s < n_s - 1:
                        nc.vector.tensor_add(
                            acc[:, c0:512], acc[:, c0:512], pt[:, c0:512]
                        )
                    else:
                        pt_last, c0_last = pt, c0
                    nc.tensor.matmul(
                        po[:, c0:512],
                        v_sb[:, s, 128 * h : 128 * (h + 1)],
                        pt[:, c0:512],
                        start=(s == 0),
                        stop=(s == n_s - 1),
                    )
                # denominator: broadcast partition-sums of acc; the last exp
                # tile skips the DVE chain and folds in via a 2nd accumulation
                rbc = ps_r.tile([128, 512], F32, tag="rbc", name="rbc")
                nc.tensor.matmul(
                    rbc, ones128, acc, start=True, stop=False, skip_group_check=True
                )
                nc.tensor.matmul(
                    rbc[:, c0_last:512],
                    ones128,
                    pt_last[:, c0_last:512],
                    start=False,
                    stop=True,
                    skip_group_check=True,
                )
                rbs = sc_pool.tile([128, 512], F32, tag="rbs", bufs=2)
                nc.vector.reciprocal_approx_fast(out=rbs, in_=rbc)
                nc.vector.tensor_mul(outT[h][:, tsl], po, rbs)

            if h + 1 < HL:
                wsem_sb = wqk_pool.tile([128, KT, 128], BF, tag="wsem")
                wgeo_sb = wqk_pool.tile([128, KT, 128], BF, tag="wgeo")
                nc.sync.dma_start(out=wsem_sb, in_=wsem_d[h + 1])
                nc.sync.dma_start(out=wgeo_sb, in_=wgeo_d[h + 1])
                if h == 2:  # out-proj weights: loads due ~250us, issue late
                    for hh in range(HL):
                        nc.sync.dma_start(out=wo_sb[hh], in_=wo_d[hh])
                qstk = qk_pool.tile([128, T], BF, tag="qstk")
                kstk = qk_pool.tile([128, T], BF, tag="kstk")
                for j in range(TC):
                    proj_chunk(h + 1, j, wsem_sb, wgeo_sb, qstk, kstk)

        # ---- out-projection: y[t, e] = sum_h outT_h^T @ wo_h ----
        for i in range(TT):
            ysb = y_pool.tile([128, D], BF, tag="ysb")
            for ec in range(D // 512):
                py = ps_big.tile([128, 512], F32, tag="big", name="py")
                for h in range(HL):
                    nc.tensor.matmul(
                        py,
                        outT[h][:, 128 * i : 128 * (i + 1)],
                        wo_sb[h][:, 512 * ec : 512 * (ec + 1)],
                        start=(h == 0),
                        stop=(h == HL - 1),
                    )
                if ec % 2 == 0:
                    nc.scalar.copy(ysb[:, 512 * ec : 512 * (ec + 1)], py)
                else:
                    nc.vector.tensor_copy(ysb[:, 512 * ec : 512 * (ec + 1)], py)
            nc.scalar.dma_start(out=y_d[128 * i : 128 * (i + 1), :], in_=ysb)

    nc.finalize()
    return nc


def _host_prep(x, w_q_sem, w_k_sem, w_q_geo, w_k_geo, w_v, w_out, gate_logit, gate_w):
    """Build the 8 per-core input maps (all numpy, bf16 where matmul-bound)."""
    half = GEO_HD // 2  # 32
    inv_freq = 1.0 / (ROPE_BASE ** (np.arange(half, dtype=np.float64) / half))
    pos = np.arange(T, dtype=np.float64)
    ang = pos[None, :] * inv_freq[:, None]  # (32, T)
    cos, sin = np.cos(ang), np.sin(ang)
    crep = np.empty((128, T), dtype=NPBF)
    srep = np.empty((128, T), dtype=NPBF)
    for b0 in (0, 64):
        crep[b0 : b0 + 32] = cos
        crep[b0 + 32 : b0 + 64] = cos
        srep[b0 : b0 + 32] = sin  # sw[0:32]=m2[32:64] needs +sin here
        srep[b0 + 32 : b0 + 64] = -sin  # sw[32:64]=m2[0:32] needs -sin here
    # rot[0:32] = p[0:32]*cos - p[32:64]*sin = m1[0:32] + (p[32:64]*srep[32:64])
    # rot[32:64] = p[32:64]*cos + p[0:32]*sin = m1[32:64] + (p[0:32]*srep[0:32])
    # (sw swaps the 32-blocks, so srep rows carry the sign of the *destination*)

    p_i = np.arange(128)
    mbias = np.where(p_i[:, None] <= p_i[None, :], 0.0, NEG_INF).astype(NPBF)
    ident = np.eye(128, dtype=NPBF)
    selc = np.zeros((128, HL * 128), dtype=NPBF)
    for h in range(HL):
        selc[h, 128 * h : 128 * h + 64] = 1.0
        selc[HL + h, 128 * h + 64 : 128 * h + 128] = 1.0
    cpack = np.concatenate(
        [crep, srep, ident, mbias, selc], axis=1
    )  # (128, 2T+256+512)

    def stack_heads(wa, wb):
        # per-head (D, 128) = [wa_head | wb_head], as (128, KT, 128) lhsT tiles
        out = []
        for h in range(H):
            blk = np.concatenate(
                [wa[:, 64 * h : 64 * (h + 1)], wb[:, 64 * h : 64 * (h + 1)]], axis=1
            )
            out.append(
                np.ascontiguousarray(
                    blk.reshape(KT, 128, 128).transpose(1, 0, 2)
                ).astype(NPBF)
            )
        return out

    wsem_all = stack_heads(w_q_sem, w_k_sem)  # [q_sem | k_sem]
    wgeo_all = stack_heads(w_k_geo, w_q_geo)  # [k_geo | q_geo]

    xt_by_b = [
        np.ascontiguousarray(
            x[b].T.reshape(KT, 128, TC, 512).transpose(1, 2, 0, 3)
        ).astype(NPBF)
        for b in range(B)
    ]  # (128, TC, KT, 512): [p, j, k, c] = xT[128k+p, 512j+c]

    in_maps = []
    for core in range(8):
        b, hg = core // 4, core % 4
        heads = range(4 * hg, 4 * hg + 4)
        wsem = np.stack([wsem_all[h] for h in heads])
        wgeo = np.stack([wgeo_all[h] for h in heads])
        wv = np.ascontiguousarray(
            w_v[:, CL * hg : CL * (hg + 1)].reshape(KT, 128, CL).transpose(1, 0, 2)
        ).astype(NPBF)
        wo = w_out[CL * hg : CL * (hg + 1), :].reshape(HL, 128, D).astype(NPBF)
        gwl = gate_w[:, 4 * hg : 4 * hg + 4]  # (D, 4)
        gw2 = np.concatenate([gwl, gwl], axis=1)  # (D, 8) duplicated
        wg = np.ascontiguousarray(
            gw2.reshape(KT, 128, 2 * HL).transpose(1, 0, 2)
        ).astype(NPBF)
        gll = gate_logit[4 * hg : 4 * hg + 4]
        glog = np.concatenate([gll, gll]).astype(np.float32)
        gsv = np.array([0.125] * HL + [-0.125] * HL, dtype=np.float32)
        gbv = np.array([0.0] * HL + [0.125] * HL, dtype=np.float32)
        gpack = np.ascontiguousarray(np.stack([glog, gsv, gbv], axis=1))  # (8, 3)
        in_maps.append(
            {
                "xt": xt_by_b[b],
                "wsem": wsem,
                "wgeo": wgeo,
                "wv": wv,
                "wo": np.ascontiguousarray(wo),
                "wg": wg,
                "cpack": cpack,
                "gpack": gpack,
            }
        )
    return in_maps


def _run(inputs, trace=False):
    global _CACHED_NC
    if _CACHED_NC is None:
        _CACHED_NC = _build_nc()
    in_maps = _host_prep(**{k: np.asarray(v) for k, v in inputs.items()})
    res = run_bass_kernel_spmd(
        _CACHED_NC, in_maps, core_ids=list(range(8)), trace=trace
    )
    y = np.zeros((B, T, D), dtype=np.float32)
    for core in range(8):
        y[core // 4] += res.results[core]["y"].astype(np.float32)
    return y, res


def kernel(**inputs) -> np.ndarray:
    y, _ = _run(inputs, trace=False)
    return y


# revision 18
# speedup vs baseline: 1.2249x; 1.0219x over previous
"""Gated dual-score (semantic+geometric/RoPE) causal attention layer on 8 TRN2 cores.

Sharding: data-parallel over batch (2) x tensor-parallel over heads (16 -> 4/core).
Core i: batch b = i // 4, heads hg = i % 4 -> heads [4*hg, 4*hg+4).
Each core computes a partial y (its heads' contribution, its batch); the host
sums the 4 partials per batch (the "all-reduce" of the row-sharded out-proj).

On-device layout: all projections consume xT (d on partitions, t free) and
produce qT/kT in (d, t) layout. Scores are computed transposed (s on
partitions, t free) so P@V consumes the exp tile directly with V in natural
(t, dv) layout. Key structure choices (all aimed at keeping the PE array,
the bottleneck at ~88% busy, free of non-matmul work):
  - Projections are stacked per head as [q_sem|k_sem] and [k_geo|q_geo] so
    RoPE/gating DVE ops run on full 128-partition tiles (q and k together).
  - The causal mask is folded into the score matmul as a second accumulated
    matmul adding a -1e9 upper-triangular constant (no DVE in exp->PV path).
  - The softmax denominator is accumulated on DVE (bf16 adds of exp tiles)
    and turned into a broadcast row-sum by ONE ones(128x128) matmul per
    chunk, replacing per-tile ones-vector matmuls on the PE.
  - x / weights stream via per-(chunk,k) contiguous DMAs ordered by first
    use on the sync ring; wv/wo/y-stores use the scalar (ACT) ring.
Softmax skips max-subtraction (scores are O(5) by construction).
Compute dtype bf16 (fp32 matmul costs 4x cycles on TRN2), fp32 accumulation.
"""

import sys
from contextlib import ExitStack

import numpy as np

sys.path.insert(0, "/opt/trn_rl_repo")

import ml_dtypes  # noqa: E402

import concourse.bass as bass  # noqa: E402
from concourse import bacc  # noqa: E402
import concourse.mybir as mybir  # noqa: E402
import concourse.tile as tile  # noqa: E402
from concourse.bass_utils import run_bass_kernel_spmd  # noqa: E402

B, T, D, H = 2, 2048, 2048, 16
SEM_HD = GEO_HD = 64
V_HD = 128
HL = 4  # heads per core
CL = HL * V_HD  # local v-dim (512)
ROPE_BASE = 10000.0
NEG_INF = -1e9

KT = D // 128  # 16 k-tiles over the contraction dim
TT = T // 128  # 16 token tiles of 128
TC = T // 512  # 4 token chunks of 512
BF = mybir.dt.bfloat16
F32 = mybir.dt.float32
NPBF = ml_dtypes.bfloat16

_CACHED_NC = None


def _build_nc():
    nc = bacc.Bacc()

    # cpack columns: crep | srep | ident | mbias | selc (rows 0:8)
    CP = 2 * T + 128 + 128 + HL * 128
    xt_d = nc.declare_dram_parameter("xt", [128, TC, KT, 512], BF, isOutput=False)
    wsem_d = nc.declare_dram_parameter("wsem", [HL, 128, KT, 128], BF, isOutput=False)
    wgeo_d = nc.declare_dram_parameter("wgeo", [HL, 128, KT, 128], BF, isOutput=False)
    wv_d = nc.declare_dram_parameter("wv", [128, KT, CL], BF, isOutput=False)
    wo_d = nc.declare_dram_parameter("wo", [HL, 128, D], BF, isOutput=False)
    wg_d = nc.declare_dram_parameter("wg", [128, KT, 2 * HL], BF, isOutput=False)
    cpack_d = nc.declare_dram_parameter("cpack", [128, CP], BF, isOutput=False)
    gpack_d = nc.declare_dram_parameter("gpack", [2 * HL, 3], F32, isOutput=False)
    y_d = nc.declare_dram_parameter("y", [T, D], BF, isOutput=True)

    with tile.TileContext(nc) as tc, ExitStack() as ctx:
        singles = ctx.enter_context(tc.tile_pool(name="singles", bufs=1))
        xpool = ctx.enter_context(tc.tile_pool(name="xpool", bufs=1))
        vpool = ctx.enter_context(tc.tile_pool(name="vpool", bufs=1))
        wqk_pool = ctx.enter_context(tc.tile_pool(name="wqk", bufs=2))
        qk_pool = ctx.enter_context(tc.tile_pool(name="qk", bufs=2))
        ot_pool = ctx.enter_context(tc.tile_pool(name="ot", bufs=1))
        wo_pool = ctx.enter_context(tc.tile_pool(name="wo", bufs=1))
        p_pool = ctx.enter_context(tc.tile_pool(name="pp", bufs=8))
        sc_pool = ctx.enter_context(tc.tile_pool(name="scratch", bufs=2))
        y_pool = ctx.enter_context(tc.tile_pool(name="ysb", bufs=2))

        ps_big = ctx.enter_context(tc.tile_pool(name="ps_big", bufs=3, space="PSUM"))
        ps_s = ctx.enter_context(tc.tile_pool(name="ps_s", bufs=2, space="PSUM"))
        ps_o = ctx.enter_context(tc.tile_pool(name="ps_o", bufs=2, space="PSUM"))
        ps_r = ctx.enter_context(tc.tile_pool(name="ps_r", bufs=1, space="PSUM"))

        # ---- sync ring: wg/gpack first (the first matmul needs wg), then xT
        # chunk by chunk; per-head q/k weights follow chunk 0.
        wg = singles.tile([128, KT, 2 * HL], BF)
        nc.sync.dma_start(out=wg, in_=wg_d[:])
        gpack = singles.tile([2 * HL, 3], F32)
        nc.sync.dma_start(out=gpack, in_=gpack_d[:])
        glog = gpack[:, 0:1]
        gsv = gpack[:, 1:2]  # [1/8 x4; -1/8 x4]
        gbv = gpack[:, 2:3]  # [0 x4; 1/8 x4]
        ones128 = singles.tile([128, 128], BF)
        nc.vector.memset(ones128, 1.0)

        xt = xpool.tile([128, TC, KT, 512], BF)
        for q in range(4):
            nc.sync.dma_start(out=xt[:, 0, 4 * q : 4 * q + 4], in_=xt_d[:, 0, 4 * q : 4 * q + 4])

        wsem0 = wqk_pool.tile([128, KT, 128], BF, tag="wsem", name="wsem0")
        wgeo0 = wqk_pool.tile([128, KT, 128], BF, tag="wgeo", name="wgeo0")
        nc.sync.dma_start(out=wsem0, in_=wsem_d[0])
        nc.sync.dma_start(out=wgeo0, in_=wgeo_d[0])

        # packed constants (rope tables et al) after head-0 weights: needed
        # only once head-0's first projection chain completes
        cpack = singles.tile([128, CP], BF)
        nc.sync.dma_start(out=cpack, in_=cpack_d[:])
        crep = cpack[:, 0:T]
        srep = cpack[:, T : 2 * T]
        ident = cpack[:, 2 * T : 2 * T + 128]
        mbias = cpack[:, 2 * T + 128 : 2 * T + 256]
        selc = cpack[0 : 2 * HL, 2 * T + 256 : 2 * T + 256 + HL * 128]

        for j in range(1, TC):
            nc.sync.dma_start(out=xt[:, j], in_=xt_d[:, j])

        wv_pool = ctx.enter_context(tc.tile_pool(name="wvpool", bufs=1))
        wv = wv_pool.tile([128, KT, CL], BF)
        nc.sync.dma_start(out=wv, in_=wv_d[:])
        wo_sb = [
            wo_pool.tile([128, D], BF, tag=f"wo{h}", name=f"wo{h}") for h in range(HL)
        ]

        # ---- helpers ----
        gcomb = singles.tile([2 * HL, T], BF)  # rows 0:4 g/8, rows 4:8 (1-g)/8

        def gate_chunk(j):
            tsl = slice(512 * j, 512 * (j + 1))
            pg = ps_s.tile([2 * HL, 512], F32, tag="ps", name="pg")
            for k in range(KT):
                nc.tensor.matmul(
                    pg, wg[:, k, :], xt[:, j, k, :], start=(k == 0), stop=(k == KT - 1)
                )
            gsig = sc_pool.tile([2 * HL, 512], F32, tag="gsig", bufs=1)
            nc.scalar.activation(
                gsig, pg, mybir.ActivationFunctionType.Sigmoid, bias=glog
            )
            nc.scalar.activation(
                gcomb[:, tsl],
                gsig,
                mybir.ActivationFunctionType.Identity,
                scale=gsv,
                bias=gbv,
            )

        def proj_chunk(h, j, wsem_sb, wgeo_sb, qstk, kstk):
            """QK projection for head h, token chunk j.

            p_sem rows: [q_sem(0:64) | k_sem(64:128)]
            p_geo rows: [k_geo(0:64) | q_geo(64:128)]
            gbs  rows: [g/8   (0:64) | (1-g)/8 (64:128)]  (q-side scales)
            """
            tsl = slice(512 * j, 512 * (j + 1))
            p_sem = ps_big.tile([128, 512], F32, tag="big", name="p_sem")
            p_geo = ps_big.tile([128, 512], F32, tag="big", name="p_geo")
            for k in range(KT):
                nc.tensor.matmul(
                    p_sem, wsem_sb[:, k, :], xt[:, j, k, :],
                    start=(k == 0), stop=(k == KT - 1),
                )
            for k in range(KT):
                nc.tensor.matmul(
                    p_geo, wgeo_sb[:, k, :], xt[:, j, k, :],
                    start=(k == 0), stop=(k == KT - 1),
                )
            gbb = ps_big.tile([128, 512], F32, tag="big", name="gbb")
            nc.tensor.matmul(
                gbb, selc[:, 128 * h : 128 * (h + 1)], gcomb[:, tsl],
                start=True, stop=True,
            )
            gbs = sc_pool.tile([128, 512], BF, tag="gbs", bufs=2)
            nc.scalar.copy(gbs, gbb)

            # RoPE on the stacked geo tile (all 128 partitions per op)
            m1 = sc_pool.tile([128, 512], BF, tag="m1", bufs=2)
            m2 = sc_pool.tile([128, 512], BF, tag="m2", bufs=2)
            sw = sc_pool.tile([128, 512], BF, tag="sw", bufs=2)
            nc.vector.tensor_mul(m1, p_geo, crep[:, tsl])
            nc.vector.tensor_mul(m2, p_geo, srep[:, tsl])
            for blk in range(4):  # swap 32-row halves within each 64
                d0 = 64 * (blk // 2) + 32 * (blk % 2)
                s0 = 64 * (blk // 2) + 32 * (1 - blk % 2)
                nc.vector.tensor_copy(sw[d0 : d0 + 32, :], m2[s0 : s0 + 32, :])
            nc.vector.tensor_add(m1, m1, sw)  # m1 = rotated [k_geo | q_geo]

            # q side gets the gate scales folded in; k side is passthrough
            nc.vector.tensor_mul(qstk[0:64, tsl], p_sem[0:64, :], gbs[0:64, :])
            nc.vector.tensor_mul(qstk[64:128, tsl], m1[64:128, :], gbs[64:128, :])
            nc.vector.tensor_copy(kstk[0:64, tsl], p_sem[64:128, :])
            nc.vector.tensor_copy(kstk[64:128, tsl], m1[0:64, :])

        # ---- gate + head-0 projection, chunk by chunk (tracks DMA arrival) ----
        qstk0 = qk_pool.tile([128, T], BF, tag="qstk", name="qstk0")
        kstk0 = qk_pool.tile([128, T], BF, tag="kstk", name="kstk0")
        for j in range(TC):
            gate_chunk(j)
            proj_chunk(0, j, wsem0, wgeo0, qstk0, kstk0)

        # ---- V projection, natural (t, dv) layout ----
        v_sb = vpool.tile([128, TT, CL], BF)
        for i in range(TT):
            pv = ps_big.tile([128, CL], F32, tag="big", name="pv")
            for k in range(KT):
                nc.tensor.matmul(
                    pv,
                    xt[:, i // 4, k, 128 * (i % 4) : 128 * (i % 4 + 1)],
                    wv[:, k, :],
                    start=(k == 0),
                    stop=(k == KT - 1),
                )
            if i % 2 == 0:
                nc.scalar.copy(v_sb[:, i, :], pv)
            else:
                nc.vector.tensor_copy(v_sb[:, i, :], pv)

        # ---- per-head: attention, then next head's projection ----
        outT = [
            ot_pool.tile([128, T], BF, tag=f"ot{h}", name=f"ot{h}") for h in range(HL)
        ]
        qstk, kstk = qstk0, kstk0
        for h in range(HL):
            for j in range(TC):
                tsl = slice(512 * j, 512 * (j + 1))
                po = ps_o.tile([128, 512], F32, tag="po")
                acc = sc_pool.tile([128, 512], BF, tag="acc", bufs=2)
                n_s = 4 * (j + 1)
                for s in range(n_s):
                    dj = s - 4 * j  # >=0 on diagonal tiles
                    c0 = 128 * dj if dj >= 0 else 0
                    ssl = slice(128 * s, 128 * (s + 1))
                    ps = ps_s.tile([128, 512], F32, tag="ps", name="ps")
                    nc.tensor.matmul(
                        ps[:, c0:512],
                        kstk[:, ssl],
                        qstk[:, 512 * j + c0 : 512 * (j + 1)],
                        start=True,
                        stop=(dj < 0),
                        skip_group_check=(dj >= 0),
                    )
                    if dj >= 0:
                        # causal mask: add -1e9 upper triangle to the diag block
                        nc.tensor.matmul(
                            ps[:, c0 : c0 + 128],
                            ident,
                            mbias,
                            start=False,
                            stop=True,
                            skip_group_check=True,
                        )
                    pt = p_pool.tile([128, 512], BF, tag="pt", name="pt")
                    nc.scalar.activation(
                        pt[:, c0:512], ps[:, c0:512], mybir.ActivationFunctionType.Exp
                    )
                    if s == 0:
                        nc.vector.tensor_copy(acc, pt)
                    elif s < n_s - 1:
                        nc.vector.tensor_add(
                            acc[:, c0:512], acc[:, c0:512], pt[:, c0:512]
                        )
                    else:
                        pt_last, c0_last = pt, c0
                    nc.tensor.matmul(
                        po[:, c0:512],
                        v_sb[:, s, 128 * h : 128 * (h + 1)],
                        pt[:, c0:512],
                        start=(s == 0),
                        stop=(s == n_s - 1),
                    )
                # denominator: broadcast partition-sums of acc; the last exp
                # tile skips the DVE chain and folds in via a 2nd accumulation
                rbc = ps_r.tile([128, 512], F32, tag="rbc", name="rbc")
                nc.tensor.matmul(
                    rbc, ones128, acc, start=True, stop=False, skip_group_check=True
                )
                nc.tensor.matmul(
                    rbc[:, c0_last:512],
                    ones128,
                    pt_last[:, c0_last:512],
                    start=False,
                    stop=True,
                    skip_group_check=True,
                )
                rbs = sc_pool.tile([128, 512], F32, tag="rbs", bufs=2)
                nc.vector.reciprocal_approx_fast(out=rbs, in_=rbc)
                nc.vector.tensor_mul(outT[h][:, tsl], po, rbs)

            if h + 1 < HL:
                wsem_sb = wqk_pool.tile([128, KT, 128], BF, tag="wsem")
                wgeo_sb = wqk_pool.tile([128, KT, 128], BF, tag="wgeo")
                nc.sync.dma_start(out=wsem_sb, in_=wsem_d[h + 1])
                nc.sync.dma_start(out=wgeo_sb, in_=wgeo_d[h + 1])
                if h == 2:  # out-proj weights: loads due ~250us, issue late
                    for hh in range(HL):
                        nc.sync.dma_start(out=wo_sb[hh], in_=wo_d[hh])
                qstk = qk_pool.tile([128, T], BF, tag="qstk")
                kstk = qk_pool.tile([128, T], BF, tag="kstk")
                for j in range(TC):
                    proj_chunk(h + 1, j, wsem_sb, wgeo_sb, qstk, kstk)

        # ---- out-projection: y[t, e] = sum_h outT_h^T @ wo_h ----
        for i in range(TT):
            ysb = y_pool.tile([128, D], BF, tag="ysb")
            for ec in range(D // 512):
                py = ps_big.tile([128, 512], F32, tag="big", name="py")
                for h in range(HL):
                    nc.tensor.matmul(
                        py,
                        outT[h][:, 128 * i : 128 * (i + 1)],
                        wo_sb[h][:, 512 * ec : 512 * (ec + 1)],
                        start=(h == 0),
                        stop=(h == HL - 1),
                    )
                if ec % 2 == 0:
                    nc.scalar.copy(ysb[:, 512 * ec : 512 * (ec + 1)], py)
                else:
                    nc.vector.tensor_copy(ysb[:, 512 * ec : 512 * (ec + 1)], py)
            nc.scalar.dma_start(out=y_d[128 * i : 128 * (i + 1), :], in_=ysb)

    nc.finalize()
    return nc


def _host_prep(x, w_q_sem, w_k_sem, w_q_geo, w_k_geo, w_v, w_out, gate_logit, gate_w):
    """Build the 8 per-core input maps (all numpy, bf16 where matmul-bound)."""
    half = GEO_HD // 2  # 32
    inv_freq = 1.0 / (ROPE_BASE ** (np.arange(half, dtype=np.float64) / half))
    pos = np.arange(T, dtype=np.float64)
    ang = pos[None, :] * inv_freq[:, None]  # (32, T)
    cos, sin = np.cos(ang), np.sin(ang)
    crep = np.empty((128, T), dtype=NPBF)
    srep = np.empty((128, T), dtype=NPBF)
    for b0 in (0, 64):
        crep[b0 : b0 + 32] = cos
        crep[b0 + 32 : b0 + 64] = cos
        srep[b0 : b0 + 32] = sin  # sw[0:32]=m2[32:64] needs +sin here
        srep[b0 + 32 : b0 + 64] = -sin  # sw[32:64]=m2[0:32] needs -sin here
    # rot[0:32] = p[0:32]*cos - p[32:64]*sin = m1[0:32] + (p[32:64]*srep[32:64])
    # rot[32:64] = p[32:64]*cos + p[0:32]*sin = m1[32:64] + (p[0:32]*srep[0:32])
    # (sw swaps the 32-blocks, so srep rows carry the sign of the *destination*)

    p_i = np.arange(128)
    mbias = np.where(p_i[:, None] <= p_i[None, :], 0.0, NEG_INF).astype(NPBF)
    ident = np.eye(128, dtype=NPBF)
    selc = np.zeros((128, HL * 128), dtype=NPBF)
    for h in range(HL):
        selc[h, 128 * h : 128 * h + 64] = 1.0
        selc[HL + h, 128 * h + 64 : 128 * h + 128] = 1.0
    cpack = np.concatenate(
        [crep, srep, ident, mbias, selc], axis=1
    )  # (128, 2T+256+512)

    def stack_heads(wa, wb):
        # per-head (D, 128) = [wa_head | wb_head], as (128, KT, 128) lhsT tiles
        out = []
        for h in range(H):
            blk = np.concatenate(
                [wa[:, 64 * h : 64 * (h + 1)], wb[:, 64 * h : 64 * (h + 1)]], axis=1
            )
            out.append(
                np.ascontiguousarray(
                    blk.reshape(KT, 128, 128).transpose(1, 0, 2)
                ).astype(NPBF)
            )
        return out

    wsem_all = stack_heads(w_q_sem, w_k_sem)  # [q_sem | k_sem]
    wgeo_all = stack_heads(w_k_geo, w_q_geo)  # [k_geo | q_geo]

    xt_by_b = [
        np.ascontiguousarray(
            x[b].T.reshape(KT, 128, TC, 512).transpose(1, 2, 0, 3)
        ).astype(NPBF)
        for b in range(B)
    ]  # (128, TC, KT, 512): [p, j, k, c] = xT[128k+p, 512j+c]

    in_maps = []
    for core in range(8):
        b, hg = core // 4, core % 4
        heads = range(4 * hg, 4 * hg + 4)
        wsem = np.stack([wsem_all[h] for h in heads])
        wgeo = np.stack([wgeo_all[h] for h in heads])
        wv = np.ascontiguousarray(
            w_v[:, CL * hg : CL * (hg + 1)].reshape(KT, 128, CL).transpose(1, 0, 2)
        ).astype(NPBF)
        wo = w_out[CL * hg : CL * (hg + 1), :].reshape(HL, 128, D).astype(NPBF)
        gwl = gate_w[:, 4 * hg : 4 * hg + 4]  # (D, 4)
        gw2 = np.concatenate([gwl, gwl], axis=1)  # (D, 8) duplicated
        wg = np.ascontiguousarray(
            gw2.reshape(KT, 128, 2 * HL).transpose(1, 0, 2)
        ).astype(NPBF)
        gll = gate_logit[4 * hg : 4 * hg + 4]
        glog = np.concatenate([gll, gll]).astype(np.float32)
        gsv = np.array([0.125] * HL + [-0.125] * HL, dtype=np.float32)
        gbv = np.array([0.0] * HL + [0.125] * HL, dtype=np.float32)
        gpack = np.ascontiguousarray(np.stack([glog, gsv, gbv], axis=1))  # (8, 3)
        in_maps.append(
            {
                "xt": xt_by_b[b],
                "wsem": wsem,
                "wgeo": wgeo,
                "wv": wv,
                "wo": np.ascontiguousarray(wo),
                "wg": wg,
                "cpack": cpack,
                "gpack": gpack,
            }
        )
    return in_maps


def _run(inputs, trace=False):
    global _CACHED_NC
    if _CACHED_NC is None:
        _CACHED_NC = _build_nc()
    in_maps = _host_prep(**{k: np.asarray(v) for k, v in inputs.items()})
    res = run_bass_kernel_spmd(
        _CACHED_NC, in_maps, core_ids=list(range(8)), trace=trace
    )
    y = np.zeros((B, T, D), dtype=np.float32)
    for core in range(8):
        y[core // 4] += res.results[core]["y"].astype(np.float32)
    return y, res


def kernel(**inputs) -> np.ndarray:
    y, _ = _run(inputs, trace=False)
    return y


# revision 26
# speedup vs baseline: 1.2269x; 1.0016x over previous
"""Gated dual-score (semantic+geometric/RoPE) causal attention layer on 8 TRN2 cores.

Sharding: data-parallel over batch (2) x tensor-parallel over heads (16 -> 4/core).
Core i: batch b = i // 4, heads hg = i % 4 -> heads [4*hg, 4*hg+4).
Each core computes a partial y (its heads' contribution, its batch); the host
sums the 4 partials per batch (the "all-reduce" of the row-sharded out-proj).

On-device layout: all projections consume xT (d on partitions, t free) and
produce qT/kT in (d, t) layout. Scores are computed transposed (s on
partitions, t free) so P@V consumes the exp tile directly with V in natural
(t, dv) layout. Key structure choices (all aimed at keeping the PE array,
the bottleneck at ~88% busy, free of non-matmul work):
  - Projections are stacked per head as [q_sem|k_sem] and [k_geo|q_geo] so
    RoPE/gating DVE ops run on full 128-partition tiles (q and k together).
  - The causal mask is folded into the score matmul as a second accumulated
    matmul adding a -1e9 upper-triangular constant (no DVE in exp->PV path).
  - The softmax denominator is accumulated on DVE (bf16 adds of exp tiles)
    and turned into a broadcast row-sum by ONE ones(128x128) matmul per
    chunk, replacing per-tile ones-vector matmuls on the PE.
  - x / weights stream via per-(chunk,k) contiguous DMAs ordered by first
    use on the sync ring; wv/wo/y-stores use the scalar (ACT) ring.
Softmax skips max-subtraction (scores are O(5) by construction).
Compute dtype bf16 (fp32 matmul costs 4x cycles on TRN2), fp32 accumulation.
"""

import sys
from contextlib import ExitStack

import numpy as np

sys.path.insert(0, "/opt/trn_rl_repo")

import ml_dtypes  # noqa: E402

import concourse.bass as bass  # noqa: E402
from concourse import bacc  # noqa: E402
import concourse.mybir as mybir  # noqa: E402
import concourse.tile as tile  # noqa: E402
from concourse.bass_utils import run_bass_kernel_spmd  # noqa: E402

B, T, D, H = 2, 2048, 2048, 16
SEM_HD = GEO_HD = 64
V_HD = 128
HL = 4  # heads per core
CL = HL * V_HD  # local v-dim (512)
ROPE_BASE = 10000.0
NEG_INF = -1e9

KT = D // 128  # 16 k-tiles over the contraction dim
TT = T // 128  # 16 token tiles of 128
TC = T // 512  # 4 token chunks of 512
BF = mybir.dt.bfloat16
F32 = mybir.dt.float32
NPBF = ml_dtypes.bfloat16

_CACHED_NC = None


def _build_nc():
    nc = bacc.Bacc()

    # cpack columns: crep | srep | ident | mbias | selc (rows 0:8)
    CP = 2 * T + 128 + 128 + HL * 128
    xt_d = nc.declare_dram_parameter("xt", [128, TC, KT, 512], BF, isOutput=False)
    wqk_d = nc.declare_dram_parameter("wqk", [HL, 128, 2, KT, 128], BF, isOutput=False)
    wv_d = nc.declare_dram_parameter("wv", [128, KT, CL], BF, isOutput=False)
    wo_d = nc.declare_dram_parameter("wo", [HL, 128, D], BF, isOutput=False)
    wg_d = nc.declare_dram_parameter("wg", [128, KT, 2 * HL], BF, isOutput=False)
    cpack_d = nc.declare_dram_parameter("cpack", [128, CP], BF, isOutput=False)
    gpack_d = nc.declare_dram_parameter("gpack", [2 * HL, 3], F32, isOutput=False)
    y_d = nc.declare_dram_parameter("y", [T, D], BF, isOutput=True)

    with tile.TileContext(nc) as tc, ExitStack() as ctx:
        singles = ctx.enter_context(tc.tile_pool(name="singles", bufs=1))
        xpool = ctx.enter_context(tc.tile_pool(name="xpool", bufs=1))
        vpool = ctx.enter_context(tc.tile_pool(name="vpool", bufs=1))
        wqk_pool = ctx.enter_context(tc.tile_pool(name="wqk", bufs=2))
        qk_pool = ctx.enter_context(tc.tile_pool(name="qk", bufs=2))
        ot_pool = ctx.enter_context(tc.tile_pool(name="ot", bufs=1))
        wo_pool = ctx.enter_context(tc.tile_pool(name="wo", bufs=1))
        p_pool = ctx.enter_context(tc.tile_pool(name="pp", bufs=8))
        sc_pool = ctx.enter_context(tc.tile_pool(name="scratch", bufs=2))
        y_pool = ctx.enter_context(tc.tile_pool(name="ysb", bufs=2))

        ps_big = ctx.enter_context(tc.tile_pool(name="ps_big", bufs=3, space="PSUM"))
        ps_s = ctx.enter_context(tc.tile_pool(name="ps_s", bufs=2, space="PSUM"))
        ps_o = ctx.enter_context(tc.tile_pool(name="ps_o", bufs=2, space="PSUM"))
        ps_r = ctx.enter_context(tc.tile_pool(name="ps_r", bufs=1, space="PSUM"))

        # ---- sync ring: wg/gpack first (the first matmul needs wg), then xT
        # chunk by chunk; per-head q/k weights follow chunk 0.
        wg = singles.tile([128, KT, 2 * HL], BF)
        nc.sync.dma_start(out=wg, in_=wg_d[:])
        gpack = singles.tile([2 * HL, 3], F32)
        nc.sync.dma_start(out=gpack, in_=gpack_d[:])
        glog = gpack[:, 0:1]
        gsv = gpack[:, 1:2]  # [1/8 x4; -1/8 x4]
        gbv = gpack[:, 2:3]  # [0 x4; 1/8 x4]
        ones128 = singles.tile([128, 128], BF)
        nc.vector.memset(ones128, 1.0)

        xt = xpool.tile([128, TC, KT, 512], BF)
        for q in range(2):
            nc.sync.dma_start(out=xt[:, 0, 8 * q : 8 * q + 8], in_=xt_d[:, 0, 8 * q : 8 * q + 8])

        wqk0 = wqk_pool.tile([128, 2, KT, 128], BF, tag="wqk", name="wqk0")
        nc.sync.dma_start(out=wqk0, in_=wqk_d[0])

        # packed constants (rope tables et al) after head-0 weights: needed
        # only once head-0's first projection chain completes
        cpack = singles.tile([128, CP], BF)
        nc.sync.dma_start(out=cpack, in_=cpack_d[:])
        crep = cpack[:, 0:T]
        srep = cpack[:, T : 2 * T]
        ident = cpack[:, 2 * T : 2 * T + 128]
        mbias = cpack[:, 2 * T + 128 : 2 * T + 256]
        selc = cpack[0 : 2 * HL, 2 * T + 256 : 2 * T + 256 + HL * 128]

        for j in range(1, TC):
            nc.sync.dma_start(out=xt[:, j], in_=xt_d[:, j])

        wv_pool = ctx.enter_context(tc.tile_pool(name="wvpool", bufs=1))
        wv = wv_pool.tile([128, KT, CL], BF)
        nc.sync.dma_start(out=wv, in_=wv_d[:])
        wo_sb = [
            wo_pool.tile([128, D], BF, tag=f"wo{h}", name=f"wo{h}") for h in range(HL)
        ]

        # ---- helpers ----
        gcomb = singles.tile([2 * HL, T], BF)  # rows 0:4 g/8, rows 4:8 (1-g)/8

        def gate_chunk(j):
            tsl = slice(512 * j, 512 * (j + 1))
            pg = ps_s.tile([2 * HL, 512], F32, tag="ps", name="pg")
            for k in range(KT):
                nc.tensor.matmul(
                    pg, wg[:, k, :], xt[:, j, k, :], start=(k == 0), stop=(k == KT - 1)
                )
            gsig = sc_pool.tile([2 * HL, 512], F32, tag="gsig", bufs=1)
            nc.scalar.activation(
                gsig, pg, mybir.ActivationFunctionType.Sigmoid, bias=glog
            )
            nc.scalar.activation(
                gcomb[:, tsl],
                gsig,
                mybir.ActivationFunctionType.Identity,
                scale=gsv,
                bias=gbv,
            )

        def proj_chunk(h, j, wqk_sb, qstk, kstk):
            wsem_sb, wgeo_sb = wqk_sb[:, 0], wqk_sb[:, 1]
            """QK projection for head h, token chunk j.

            p_sem rows: [q_sem(0:64) | k_sem(64:128)]
            p_geo rows: [k_geo(0:64) | q_geo(64:128)]
            gbs  rows: [g/8   (0:64) | (1-g)/8 (64:128)]  (q-side scales)
            """
            tsl = slice(512 * j, 512 * (j + 1))
            p_sem = ps_big.tile([128, 512], F32, tag="big", name="p_sem")
            p_geo = ps_big.tile([128, 512], F32, tag="big", name="p_geo")
            for k in range(KT):
                nc.tensor.matmul(
                    p_sem, wsem_sb[:, k, :], xt[:, j, k, :],
                    start=(k == 0), stop=(k == KT - 1),
                )
            for k in range(KT):
                nc.tensor.matmul(
                    p_geo, wgeo_sb[:, k, :], xt[:, j, k, :],
                    start=(k == 0), stop=(k == KT - 1),
                )
            gbb = ps_big.tile([128, 512], F32, tag="big", name="gbb")
            nc.tensor.matmul(
                gbb, selc[:, 128 * h : 128 * (h + 1)], gcomb[:, tsl],
                start=True, stop=True,
            )
            gbs = sc_pool.tile([128, 512], BF, tag="gbs", bufs=2)
            nc.scalar.copy(gbs, gbb)

            # RoPE on the stacked geo tile (all 128 partitions per op)
            m1 = sc_pool.tile([128, 512], BF, tag="m1", bufs=2)
            m2 = sc_pool.tile([128, 512], BF, tag="m2", bufs=2)
            sw = sc_pool.tile([128, 512], BF, tag="sw", bufs=2)
            nc.vector.tensor_mul(m1, p_geo, crep[:, tsl])
            nc.vector.tensor_mul(m2, p_geo, srep[:, tsl])
            for blk in range(4):  # swap 32-row halves within each 64
                d0 = 64 * (blk // 2) + 32 * (blk % 2)
                s0 = 64 * (blk // 2) + 32 * (1 - blk % 2)
                nc.vector.tensor_copy(sw[d0 : d0 + 32, :], m2[s0 : s0 + 32, :])
            nc.vector.tensor_add(m1, m1, sw)  # m1 = rotated [k_geo | q_geo]

            # q side gets the gate scales folded in; k side is passthrough
            nc.vector.tensor_mul(qstk[0:64, tsl], p_sem[0:64, :], gbs[0:64, :])
            nc.vector.tensor_mul(qstk[64:128, tsl], m1[64:128, :], gbs[64:128, :])
            nc.vector.tensor_copy(kstk[0:64, tsl], p_sem[64:128, :])
            nc.vector.tensor_copy(kstk[64:128, tsl], m1[0:64, :])

        # ---- gate + head-0 projection, chunk by chunk (tracks DMA arrival) ----
        qstk0 = qk_pool.tile([128, T], BF, tag="qstk", name="qstk0")
        kstk0 = qk_pool.tile([128, T], BF, tag="kstk", name="kstk0")
        for j in range(TC):
            gate_chunk(j)
            proj_chunk(0, j, wqk0, qstk0, kstk0)

        # ---- V projection, natural (t, dv) layout ----
        v_sb = vpool.tile([128, TT, CL], BF)
        for i in range(TT):
            pv = ps_big.tile([128, CL], F32, tag="big", name="pv")
            for k in range(KT):
                nc.tensor.matmul(
                    pv,
                    xt[:, i // 4, k, 128 * (i % 4) : 128 * (i % 4 + 1)],
                    wv[:, k, :],
                    start=(k == 0),
                    stop=(k == KT - 1),
                )
            if i % 2 == 0:
                nc.scalar.copy(v_sb[:, i, :], pv)
            else:
                nc.vector.tensor_copy(v_sb[:, i, :], pv)

        # ---- per-head: attention, then next head's projection ----
        outT = [
            ot_pool.tile([128, T], BF, tag=f"ot{h}", name=f"ot{h}") for h in range(HL)
        ]
        qstk, kstk = qstk0, kstk0
        for h in range(HL):
            for j in range(TC):
                tsl = slice(512 * j, 512 * (j + 1))
                po = ps_o.tile([128, 512], F32, tag="po")
                acc = sc_pool.tile([128, 512], BF, tag="acc", bufs=2)
                n_s = 4 * (j + 1)
                for s in range(n_s):
                    dj = s - 4 * j  # >=0 on diagonal tiles
                    c0 = 128 * dj if dj >= 0 else 0
                    ssl = slice(128 * s, 128 * (s + 1))
                    ps = ps_s.tile([128, 512], F32, tag="ps", name="ps")
                    nc.tensor.matmul(
                        ps[:, c0:512],
                        kstk[:, ssl],
                        qstk[:, 512 * j + c0 : 512 * (j + 1)],
                        start=True,
                        stop=(dj < 0),
                        skip_group_check=(dj >= 0),
                    )
                    if dj >= 0:
                        # causal mask: add -1e9 upper triangle to the diag block
                        nc.tensor.matmul(
                            ps[:, c0 : c0 + 128],
                            ident,
                            mbias,
                            start=False,
                            stop=True,
                            skip_group_check=True,
                        )
                    pt = p_pool.tile([128, 512], BF, tag="pt", name="pt")
                    nc.scalar.activation(
                        pt[:, c0:512], ps[:, c0:512], mybir.ActivationFunctionType.Exp
                    )
                    if s == 0:
                        nc.vector.tensor_copy(acc, pt)
                    elif s < n_s - 1:
                        nc.vector.tensor_add(
                            acc[:, c0:512], acc[:, c0:512], pt[:, c0:512]
                        )
                    else:
                        pt_last, c0_last = pt, c0
                    nc.tensor.matmul(
                        po[:, c0:512],
                        v_sb[:, s, 128 * h : 128 * (h + 1)],
                        pt[:, c0:512],
                        start=(s == 0),
                        stop=(s == n_s - 1),
                    )
                # denominator: broadcast partition-sums of acc; the last exp
                # tile skips the DVE chain and folds in via a 2nd accumulation
                rbc = ps_r.tile([128, 512], F32, tag="rbc", name="rbc")
                nc.tensor.matmul(
                    rbc, ones128, acc, start=True, stop=False, skip_group_check=True
                )
                nc.tensor.matmul(
                    rbc[:, c0_last:512],
                    ones128,
                    pt_last[:, c0_last:512],
                    start=False,
                    stop=True,
                    skip_group_check=True,
                )
                rbs = sc_pool.tile([128, 512], F32, tag="rbs", bufs=2)
                nc.vector.reciprocal_approx_fast(out=rbs, in_=rbc)
                nc.vector.tensor_mul(outT[h][:, tsl], po, rbs)

            if h + 1 < HL:
                wqk_sb = wqk_pool.tile([128, 2, KT, 128], BF, tag="wqk")
                nc.sync.dma_start(out=wqk_sb, in_=wqk_d[h + 1])
                if h == 2:  # out-proj weights: loads due ~250us, issue late
                    for hh in range(HL):
                        nc.sync.dma_start(out=wo_sb[hh], in_=wo_d[hh])
                qstk = qk_pool.tile([128, T], BF, tag="qstk")
                kstk = qk_pool.tile([128, T], BF, tag="kstk")
                for j in range(TC):
                    proj_chunk(h + 1, j, wqk_sb, qstk, kstk)

        # ---- out-projection: y[t, e] = sum_h outT_h^T @ wo_h ----
        for i in range(TT):
            ysb = y_pool.tile([128, D], BF, tag="ysb")
            for ec in range(D // 512):
                py = ps_big.tile([128, 512], F32, tag="big", name="py")
                for h in range(HL):
                    nc.tensor.matmul(
                        py,
                        outT[h][:, 128 * i : 128 * (i + 1)],
                        wo_sb[h][:, 512 * ec : 512 * (ec + 1)],
                        start=(h == 0),
                        stop=(h == HL - 1),
                    )
                if ec % 2 == 0:
                    nc.scalar.copy(ysb[:, 512 * ec : 512 * (ec + 1)], py)
                else:
                    nc.vector.tensor_copy(ysb[:, 512 * ec : 512 * (ec + 1)], py)
            nc.scalar.dma_start(out=y_d[128 * i : 128 * (i + 1), :], in_=ysb)

    nc.finalize()
    return nc


def _host_prep(x, w_q_sem, w_k_sem, w_q_geo, w_k_geo, w_v, w_out, gate_logit, gate_w):
    """Build the 8 per-core input maps (all numpy, bf16 where matmul-bound)."""
    half = GEO_HD // 2  # 32
    inv_freq = 1.0 / (ROPE_BASE ** (np.arange(half, dtype=np.float64) / half))
    pos = np.arange(T, dtype=np.float64)
    ang = pos[None, :] * inv_freq[:, None]  # (32, T)
    cos, sin = np.cos(ang), np.sin(ang)
    crep = np.empty((128, T), dtype=NPBF)
    srep = np.empty((128, T), dtype=NPBF)
    for b0 in (0, 64):
        crep[b0 : b0 + 32] = cos
        crep[b0 + 32 : b0 + 64] = cos
        srep[b0 : b0 + 32] = sin  # sw[0:32]=m2[32:64] needs +sin here
        srep[b0 + 32 : b0 + 64] = -sin  # sw[32:64]=m2[0:32] needs -sin here
    # rot[0:32] = p[0:32]*cos - p[32:64]*sin = m1[0:32] + (p[32:64]*srep[32:64])
    # rot[32:64] = p[32:64]*cos + p[0:32]*sin = m1[32:64] + (p[0:32]*srep[0:32])
    # (sw swaps the 32-blocks, so srep rows carry the sign of the *destination*)

    p_i = np.arange(128)
    mbias = np.where(p_i[:, None] <= p_i[None, :], 0.0, NEG_INF).astype(NPBF)
    ident = np.eye(128, dtype=NPBF)
    selc = np.zeros((128, HL * 128), dtype=NPBF)
    for h in range(HL):
        selc[h, 128 * h : 128 * h + 64] = 1.0
        selc[HL + h, 128 * h + 64 : 128 * h + 128] = 1.0
    cpack = np.concatenate(
        [crep, srep, ident, mbias, selc], axis=1
    )  # (128, 2T+256+512)

    def stack_heads(wa, wb):
        # per-head (D, 128) = [wa_head | wb_head], as (128, KT, 128) lhsT tiles
        out = []
        for h in range(H):
            blk = np.concatenate(
                [wa[:, 64 * h : 64 * (h + 1)], wb[:, 64 * h : 64 * (h + 1)]], axis=1
            )
            out.append(
                np.ascontiguousarray(
                    blk.reshape(KT, 128, 128).transpose(1, 0, 2)
                ).astype(NPBF)
            )
        return out

    wsem_all = stack_heads(w_q_sem, w_k_sem)  # [q_sem | k_sem]
    wgeo_all = stack_heads(w_k_geo, w_q_geo)  # [k_geo | q_geo]
    wqk_all = [
        np.ascontiguousarray(np.stack([ws, wgg], axis=1))  # (128, 2, KT, 128)
        for ws, wgg in zip(wsem_all, wgeo_all)
    ]

    xt_by_b = [
        np.ascontiguousarray(
            x[b].T.reshape(KT, 128, TC, 512).transpose(1, 2, 0, 3)
        ).astype(NPBF)
        for b in range(B)
    ]  # (128, TC, KT, 512): [p, j, k, c] = xT[128k+p, 512j+c]

    in_maps = []
    for core in range(8):
        b, hg = core // 4, core % 4
        heads = range(4 * hg, 4 * hg + 4)
        wqk = np.stack([wqk_all[h] for h in heads])
        wv = np.ascontiguousarray(
            w_v[:, CL * hg : CL * (hg + 1)].reshape(KT, 128, CL).transpose(1, 0, 2)
        ).astype(NPBF)
        wo = w_out[CL * hg : CL * (hg + 1), :].reshape(HL, 128, D).astype(NPBF)
        gwl = gate_w[:, 4 * hg : 4 * hg + 4]  # (D, 4)
        gw2 = np.concatenate([gwl, gwl], axis=1)  # (D, 8) duplicated
        wg = np.ascontiguousarray(
            gw2.reshape(KT, 128, 2 * HL).transpose(1, 0, 2)
        ).astype(NPBF)
        gll = gate_logit[4 * hg : 4 * hg + 4]
        glog = np.concatenate([gll, gll]).astype(np.float32)
        gsv = np.array([0.125] * HL + [-0.125] * HL, dtype=np.float32)
        gbv = np.array([0.0] * HL + [0.125] * HL, dtype=np.float32)
        gpack = np.ascontiguousarray(np.stack([glog, gsv, gbv], axis=1))  # (8, 3)
        in_maps.append(
            {
                "xt": xt_by_b[b],
                "wqk": wqk,
                "wv": wv,
                "wo": np.ascontiguousarray(wo),
                "wg": wg,
                "cpack": cpack,
                "gpack": gpack,
            }
        )
    return in_maps


def _run(inputs, trace=False):
    global _CACHED_NC
    if _CACHED_NC is None:
        _CACHED_NC = _build_nc()
    in_maps = _host_prep(**{k: np.asarray(v) for k, v in inputs.items()})
    res = run_bass_kernel_spmd(
        _CACHED_NC, in_maps, core_ids=list(range(8)), trace=trace
    )
    y = np.zeros((B, T, D), dtype=np.float32)
    for core in range(8):
        y[core // 4] += res.results[core]["y"].astype(np.float32)
    return y, res


def kernel(**inputs) -> np.ndarray:
    y, _ = _run(inputs, trace=False)
    return y
